# revision 6
# baseline (speedup 1.0000x reference)
"""MultiHeadDiffAttention TRN2 kernel, v2 (pipelined).

Sharding: 8 cores = 2 batches x 4 head-pairs (same as v1). Core c handles
batch c//4 and heads {2g, 2g+1}, g = c%4; its 128 channels form one
GroupNorm group. Final projection is a partial sum over the core's
channels; the host adds 4 partials per batch plus the bias.

v2 structure (vs v1):
  - inputs/weights/outputs in bf16 (halves DMA bytes); attention math in
    fp32r, projections/final matmul in bf16
  - x DMA split into 4 chunks so projections start ~4x earlier
  - ALL psum use during proj+attention goes through two pools that coexist
    within the 8 banks: sc (2 tiles x 2 banks, double-buffered) and av
    (1 x 4 banks)
  - attention pipelined: per key-block, score MMs -> exp -> (deferred one
    group) attn@V MMs, so PE never waits on the activation engine
  - loops ordered h-outer/attn-inner; the diff-attn combine and GroupNorm
    stats for head-pair h run on DVE/Pool while h+1's attention occupies
    PE/ACT
  - exp groups are [128, 1024] (2 key-blocks x 512 queries... actually
    1 key-block x 2 query-chunks, keeping the stationary operand resident
    across consecutive matmuls)
"""

import sys

sys.path.insert(0, "/opt/trn_rl_repo")

import numpy as np
from collections import deque

import concourse.bacc as bacc
import concourse.mybir as mybir
import concourse.tile as tile
from concourse.masks import make_identity
from concourse.bass_utils import run_bass_kernel_spmd

B, S, D = 2, 2048, 512
H = 8
HD = D // H          # 64
CH = 2 * HD          # 128 channels per core (one GroupNorm group)
LAMBDA_INIT = 0.2
EPS = 1e-5
N_CORES = 8

QB = 512             # query chunk (psum bank)
NQB = S // QB        # 4
KB = 128             # key block
NKB = S // KB        # 16
SB = 128             # seq block for final matmul
NSB = S // SB        # 16

F32 = mybir.dt.float32
F32R = mybir.dt.float32r
BF16 = mybir.dt.bfloat16
NWEIGHTS = 5
WIDX = {"q1": 0, "k1": 1, "q2": 2, "k2": 3, "v": 4}

_CACHE = {}

from contextlib import nullcontext


def build_program(repeats=1, hw_loop=False):
    nc = bacc.Bacc("TRN2", target_bir_lowering=False, debug=False)

    # ---- external I/O (packed per-partition-contiguous host layouts) ----
    # xp[p, c*S + s] = x[b, s, 128c+p]              (bf16)
    d_xp = nc.declare_dram_parameter("xp", [128, 4 * S], BF16, isOutput=False)
    # wp[p, w*512 + c*128 + m] = W_w[ch0+m, 128c+p]; then owT[p, d] tail
    d_wp = nc.declare_dram_parameter("wp", [128, NWEIGHTS * 512 + D],
                                     BF16, isOutput=False)
    # cp[p, :] = [k1b, k2b, gnw, gnb, neglam0, neglam1]
    d_cp = nc.declare_dram_parameter("cp", [CH, 6], F32, isOutput=False)
    # yp[p, sb*D + d] = y_part[128*sb+p, d]         (bf16)
    d_y = nc.declare_dram_parameter("y_part", [SB, NSB * D], BF16,
                                    isOutput=True)
    # yb[0, d] = (gn_b_eff . owT)[d] — constant row added host-side
    d_yb = nc.declare_dram_parameter("yb", [1, D], F32, isOutput=True)

    with tile.TileContext(nc) as tc:
     with (tc.For_i(0, repeats) if hw_loop else nullcontext()):
      for _rep in range(1 if hw_loop else repeats):
        with (
            tc.tile_pool(name="consts", bufs=1) as consts,
            tc.tile_pool(name="qk", bufs=1) as qk_pool,
            tc.tile_pool(name="vaug", bufs=1) as vaug_pool,
            tc.tile_pool(name="xtp", bufs=1) as xt_pool,
            tc.tile_pool(name="upool", bufs=9) as u_pool,
            tc.tile_pool(name="ubig", bufs=1) as ubig_pool,
            tc.tile_pool(name="opool", bufs=1) as o_pool,
            tc.tile_pool(name="small", bufs=1) as small,
        ):
            # ---- constants / packed inputs ----
            ones = consts.tile([128, 1], F32, tag="ones")
            nc.vector.memset(ones, 1.0)
            eps_t = consts.tile([1, 1], F32, tag="eps")
            nc.vector.memset(eps_t, EPS)
            cp = consts.tile([CH, 6], F32, tag="cp")
            nc.sync.dma_start(out=cp, in_=d_cp.ap())
            k1b, k2b = cp[:, 0:1], cp[:, 1:2]
            gnw, gnb = cp[:, 2:3], cp[:, 3:4]
            neglam = cp[:, 4:6]

            wt = consts.tile([128, NWEIGHTS, 4, CH], BF16, tag="wt")
            nc.sync.dma_start(
                out=wt,
                in_=d_wp.ap()[:, 0:NWEIGHTS * 512].rearrange(
                    "p (w c m) -> p w c m", w=NWEIGHTS, c=4))

            xt = xt_pool.tile([128, 4, S], BF16, tag="xt")
            for c in range(4):
                nc.sync.dma_start(out=xt[:, c, :],
                                  in_=d_xp.ap()[:, c * S:(c + 1) * S])

            owT = consts.tile([CH, D], BF16, tag="owT")
            nc.sync.dma_start(out=owT, in_=d_wp.ap()[:, NWEIGHTS * 512:])

            identf = consts.tile([SB, SB], F32, tag="identf")
            make_identity(nc, identf)
            ident = consts.tile([SB, SB], F32R, tag="ident")
            nc.vector.tensor_copy(ident, identf)

            # persistent SBUF tensors
            qk = {w: qk_pool.tile([CH, S], F32R, tag=w, name=w)
                  for w in ("q1", "k1", "q2", "k2")}
            vT = qk_pool.tile([CH, S], F32R, tag="vT")
            va = vaug_pool.tile([SB, 2 * NSB, HD + 1], F32R, tag="va")
            nc.vector.tensor_copy(va[:, :, HD:HD + 1],
                                  ones.to_broadcast((SB, 2 * NSB, 1)))
            oT = o_pool.tile([CH, S], F32, tag="oT")
            xnr = o_pool.tile([CH, S], BF16, tag="xnr")
            U = {(h, a): ubig_pool.tile([HD + 1, S], F32, tag=f"U{h}{a}",
                                         name=f"U{h}{a}")
                 for h in (0, 1) for a in (1, 2)}
            nst = 4
            BST_F = S // nst
            bstats = small.tile([CH, nst, 6], F32, tag="bstats")

            with (
                tc.tile_pool(name="sc", bufs=3, space="PSUM") as sc_pool,
                tc.tile_pool(name="avp", bufs=1, space="PSUM") as av_pool,
            ):
                # ---- projections & v-transpose as small chunks (PE
                # fillers interleaved into the first attention block) ----
                def proj_chunk(w, dst, qb, bias=None):
                    ps = sc_pool.tile([128, QB], F32, tag="sc",
                                      name=f"pj_{w}{qb}")
                    for c in range(4):
                        nc.tensor.matmul(
                            ps, wt[:, WIDX[w], c, :],
                            xt[:, c, qb * QB:(qb + 1) * QB],
                            start=(c == 0), stop=(c == 3))
                    sl = slice(qb * QB, (qb + 1) * QB)
                    if bias is not None:
                        nc.vector.tensor_scalar_add(dst[:, sl], ps, bias)
                    else:
                        nc.vector.tensor_copy(dst[:, sl], ps)

                def vtrans_chunk(grp):
                    pst = sc_pool.tile([128, 4, SB], F32R, tag="sc",
                                       name=f"pv{grp}")
                    for i in range(4):
                        sb = 4 * grp + i
                        nc.tensor.transpose(
                            pst[:, i, :], vT[:, sb * SB:(sb + 1) * SB], ident)
                    nc.vector.tensor_copy(
                        va[:, 8 * grp:8 * grp + 8, 0:HD],
                        pst.rearrange("p i (h m) -> p (i h) m", h=2))

                # ---- attention for one (h, attn): pipelined sc->exp->av,
                # processed in two query-pair chunks so the combine for a
                # chunk overlaps the next chunk's attention ----
                def attention(h, attn, fillers=None, fill_plan=None,
                              chunks=((0, 2 * QB), (2 * QB, 2 * QB))):
                    hs = slice(h * HD, (h + 1) * HD)
                    qT, kT = qk[f"q{attn}"], qk[f"k{attn}"]
                    for pair, (lo, w) in enumerate(chunks):
                        pc = slice(lo, lo + w)
                        av = av_pool.tile([HD + 1, w], F32, tag="av",
                                          name="av")
                        pending = []

                        nsub = w // QB

                        def emit_av(p, av=av, nsub=nsub):
                            ut, kb = p
                            for j in range(nsub):
                                nc.tensor.matmul(
                                    av[:, j * QB:(j + 1) * QB],
                                    va[:, 2 * kb + h, :],
                                    ut[:, j * QB:(j + 1) * QB],
                                    start=(kb == 0), stop=(kb == NKB - 1),
                                )

                        for kb in range(NKB):
                            sct = sc_pool.tile([128, w], F32, tag="sc",
                                               name="sc")
                            for j in range(nsub):
                                q0 = lo + j * QB
                                nc.tensor.matmul(
                                    sct[:, j * QB:(j + 1) * QB],
                                    kT[hs, kb * KB:(kb + 1) * KB],
                                    qT[hs, q0:q0 + QB],
                                    start=True, stop=True,
                                )
                            ut = u_pool.tile([128, w], F32R, tag="u",
                                             name="u")
                            nc.scalar.activation(
                                out=ut, in_=sct,
                                func=mybir.ActivationFunctionType.Exp,
                                scale=1.0 / (HD ** 0.5),
                            )
                            pending.append((ut, kb))
                            if fill_plan is not None:
                                nfill, nflush = fill_plan(pair, kb)
                                for _ in range(nfill):
                                    if fillers:
                                        fillers.popleft()()
                                for _ in range(nflush):
                                    if len(pending) > 1:
                                        emit_av(pending.pop(0))
                            else:
                                if len(pending) > 1:
                                    emit_av(pending.pop(0))
                        for p in pending:
                            emit_av(p)
                        nc.vector.tensor_copy(U[(h, attn)][:, pc], av)
                        if attn == 1:
                            combineA(h, lo, w)
                        else:
                            combineB(h, lo, w)

                t1s = {0: small.tile([HD, S], F32, tag="t1full",
                                     name="t1full")}

                def combineA(h, lo, w):
                    pc = slice(lo, lo + w)
                    rr = small.tile([1, w], F32, tag="rr1", name="rr")
                    nc.vector.reciprocal(out=rr, in_=U[(h, 1)][HD:HD + 1, pc])
                    rb1 = small.tile([HD, w], F32, tag="rb1", name="rb")
                    nc.gpsimd.partition_broadcast(rb1, rr)
                    nc.vector.tensor_mul(t1s[0][:, pc], U[(h, 1)][0:HD, pc],
                                         rb1)

                def combineB(h, lo, w):
                    hs = slice(h * HD, (h + 1) * HD)
                    pc = slice(lo, lo + w)
                    rr = small.tile([1, w], F32, tag="rr2", name="rr")
                    nc.vector.reciprocal(out=rr, in_=U[(h, 2)][HD:HD + 1, pc])
                    rb2 = small.tile([HD, w], F32, tag="rb2", name="rb")
                    nc.gpsimd.partition_broadcast(rb2, rr)
                    t2 = small.tile([HD, w], F32, tag="t2", name="t2")
                    nc.vector.scalar_tensor_tensor(
                        out=t2, in0=U[(h, 2)][0:HD, pc],
                        scalar=neglam[0:HD, h:h + 1], in1=rb2,
                        op0=mybir.AluOpType.mult,
                        op1=mybir.AluOpType.mult,
                    )
                    nc.vector.tensor_add(oT[hs, pc], t1s[0][:, pc], t2)
                    for ii in range(lo // BST_F, (lo + w) // BST_F):
                        nc.vector.bn_stats(
                            out=bstats[hs, ii, :],
                            in_=oT[hs, ii * BST_F:(ii + 1) * BST_F])
                    nc.vector.tensor_copy(xnr[hs, pc], oT[hs, pc])

                # ---- schedule: a 3-chunk prologue starts the exp
                # stream ~3us in; every other projection chunk and the
                # v-transpose ride inside attention(0,1) as PE fillers;
                # attn@V for early key-blocks defers until transposed V
                # chunks exist. ----
                proj_chunk("k1", qk["k1"], 0, k1b)
                proj_chunk("q1", qk["q1"], 0)
                proj_chunk("q1", qk["q1"], 1)
                fillers = deque()
                for qb in (1, 2, 3):
                    fillers.append(lambda qb=qb: proj_chunk("k1", qk["k1"],
                                                            qb, k1b))
                for qb in range(4):
                    fillers.append(lambda qb=qb: proj_chunk("v", vT, qb))
                for g in range(4):
                    fillers.append(lambda g=g: vtrans_chunk(g))
                for qb in (2, 3):
                    fillers.append(lambda qb=qb: proj_chunk("q1", qk["q1"],
                                                            qb))
                for qb in range(4):
                    fillers.append(lambda qb=qb: proj_chunk("q2", qk["q2"],
                                                            qb))
                for qb in range(4):
                    fillers.append(lambda qb=qb: proj_chunk("k2", qk["k2"],
                                                            qb, k2b))

                def fill_plan(pair, kb):
                    it = pair * NKB + kb
                    if it <= 5:
                        return (2, 0)      # 11 head fillers by iter 5
                    if it <= 7:
                        return (1, 3)      # q1 tail + start AV flush
                    if it % 2 == 0:
                        return (1, 2)      # q2/k2 spread, keep flushing
                    return (0, 2)

                attention(0, 1, fillers, fill_plan)
                attention(0, 2)
                attention(1, 1)
                attention(1, 2, chunks=((0, 2 * QB), (2 * QB, QB),
                                        (3 * QB, QB)))
                # preload the Sqrt activation table while DVE combines
                dummy = small.tile([1, 1], F32, tag="dummy")
                nc.scalar.activation(out=dummy, in_=eps_t,
                                     func=mybir.ActivationFunctionType.Sqrt,
                                     scale=1.0)

            # ---- GroupNorm global stats ----
            with tc.tile_pool(name="stp", bufs=1, space="PSUM") as stp_pool:
                mv = small.tile([CH, 2], F32, tag="mv")
                nc.vector.bn_aggr(out=mv, in_=bstats)
                s12 = small.tile([CH, 2], F32, tag="s12")
                nc.vector.tensor_copy(s12[:, 0:1], mv[:, 0:1])
                nc.vector.scalar_tensor_tensor(
                    out=s12[:, 1:2], in0=mv[:, 0:1], scalar=0.0,
                    in1=mv[:, 0:1], op0=mybir.AluOpType.add,
                    op1=mybir.AluOpType.mult)
                nc.vector.tensor_add(s12[:, 1:2], s12[:, 1:2], mv[:, 1:2])
                st = stp_pool.tile([1, 2], F32, tag="st")
                nc.tensor.matmul(st[0:1, 0:1], s12[:, 0:1], ones,
                                 start=True, stop=True)
                nc.tensor.matmul(st[0:1, 1:2], s12[:, 1:2], ones,
                                 start=True, stop=True, skip_group_check=True)
                mu_e2 = small.tile([1, 2], F32, tag="mu_e2")
                nc.vector.tensor_scalar_mul(mu_e2, st[0:1, 0:2], 1.0 / CH)
                sqm = small.tile([1, 1], F32, tag="sqm")
                nc.vector.tensor_mul(sqm, mu_e2[:, 0:1], mu_e2[:, 0:1])
                var = small.tile([1, 1], F32, tag="var")
                nc.vector.tensor_sub(var, mu_e2[:, 1:2], sqm)
                std = small.tile([1, 1], F32, tag="std")
                nc.scalar.activation(out=std, in_=var,
                                     func=mybir.ActivationFunctionType.Sqrt,
                                     bias=eps_t, scale=1.0)
                rstd = small.tile([1, 1], F32, tag="rstd")
                nc.vector.reciprocal(out=rstd, in_=std)
                murstd = small.tile([1, 2], F32, tag="murstd")
                nc.vector.tensor_copy(murstd[:, 0:1], mu_e2[:, 0:1])
                nc.vector.tensor_copy(murstd[:, 1:2], rstd)
                br = small.tile([CH, 2], F32, tag="br")
                nc.gpsimd.partition_broadcast(br, murstd)
                a_t = small.tile([CH, 1], F32, tag="a_t")
                nc.vector.tensor_mul(a_t, br[:, 1:2], gnw)
                amu = small.tile([CH, 1], F32, tag="amu")
                nc.vector.tensor_mul(amu, a_t, br[:, 0:1])
                b_t = small.tile([CH, 1], F32, tag="b_t")
                nc.vector.tensor_sub(b_t, gnb, amu)
                # fold GN affine into the output projection:
                #   y = xnr.T @ (a*owT) + (b.T @ owT)
                owTs = small.tile([CH, D], BF16, tag="owTs")
                nc.vector.tensor_scalar_mul(owTs, owT, a_t)
                b16 = small.tile([CH, 1], BF16, tag="b16")
                nc.vector.tensor_copy(b16, b_t)
                ybp = stp_pool.tile([1, D], F32, tag="ybp")
                nc.tensor.matmul(ybp, b16, owT, start=True, stop=True,
                                 skip_group_check=True)
                yb = small.tile([1, D], F32, tag="yb")
                nc.vector.tensor_copy(yb, ybp)
                nc.sync.dma_start(out=d_yb.ap(), in_=yb)

            # ---- final projection partial: y = xnr.T @ owTs ----
            with (
                tc.tile_pool(name="fin", bufs=2, space="PSUM") as fin_pool,
                tc.tile_pool(name="ytp", bufs=2) as yt_pool,
            ):
                half = NSB // 4
                for hf in range(4):
                    ps = fin_pool.tile([SB, half * D], F32, tag="fin",
                                       name="fin")
                    yt = yt_pool.tile([SB, half, D], BF16, tag="yt", name="yt")
                    for i in range(half):
                        sb = hf * half + i
                        nc.tensor.matmul(
                            ps[:, i * D:(i + 1) * D],
                            xnr[:, sb * SB:(sb + 1) * SB],
                            owTs,
                            start=True, stop=True,
                        )
                    nc.scalar.activation(
                        out=yt, in_=ps.rearrange("p (i d) -> p i d", i=half),
                        func=mybir.ActivationFunctionType.Copy, scale=1.0)
                    nc.sync.dma_start(
                        out=d_y.ap().rearrange(
                            "p (hf sb d) -> p hf sb d", hf=4, sb=half)[:, hf],
                        in_=yt)

    nc.compile()
    return nc


def _shard_inputs(inputs):
    import ml_dtypes
    bf = ml_dtypes.bfloat16
    x = np.ascontiguousarray(inputs["x"], np.float32)
    lam = (np.exp(inputs["lambda_q1"] * inputs["lambda_k1"])
           - np.exp(inputs["lambda_q2"] * inputs["lambda_k2"])
           + LAMBDA_INIT).astype(np.float32).reshape(H)
    in_maps = []
    for c in range(N_CORES):
        b, g = divmod(c, 4)
        ch = slice(CH * g, CH * (g + 1))
        # xp[p, c*S+s] = x[b, s, 128c+p]
        xp = np.ascontiguousarray(
            x[b].T.reshape(4, 128, S).transpose(1, 0, 2).reshape(128, 4 * S)
        ).astype(bf)
        wlist = []
        for W in (inputs["Q1_w"], inputs["K1_w"], inputs["Q2_w"],
                  inputs["K2_w"], inputs["V_w"]):
            wT = np.asarray(W)[ch].T  # [512, 128]
            wlist.append(np.ascontiguousarray(
                wT.reshape(4, 128, CH).transpose(1, 0, 2).reshape(128, 512)))
        owT = np.ascontiguousarray(np.asarray(inputs["out_w"])[:, ch].T)
        wp = np.concatenate(wlist + [owT], axis=1).astype(bf)
        cp = np.stack([
            np.asarray(inputs["K1_b"])[ch],
            np.asarray(inputs["K2_b"])[ch],
            np.asarray(inputs["gn_w"])[ch],
            np.asarray(inputs["gn_b"])[ch],
            np.full(CH, -lam[2 * g], np.float32),
            np.full(CH, -lam[2 * g + 1], np.float32),
        ], axis=1).astype(np.float32)
        in_maps.append({"xp": xp, "wp": wp, "cp": np.ascontiguousarray(cp)})
    return in_maps


def kernel(**inputs):
    inputs = {k: np.asarray(v) for k, v in inputs.items()}
    if "nc" not in _CACHE:
        _CACHE["nc"] = build_program()
    nc = _CACHE["nc"]
    in_maps = _shard_inputs(inputs)
    res = run_bass_kernel_spmd(nc, in_maps, list(range(N_CORES)))
    out_b = np.asarray(inputs["out_b"], np.float32)
    y = np.zeros((B, S, D), np.float32)
    for c in range(N_CORES):
        b = c // 4
        yp = res.results[c]["y_part"].astype(np.float32)
        y[b] += yp.reshape(SB, NSB, D).transpose(1, 0, 2).reshape(S, D)
        y[b] += res.results[c]["yb"].astype(np.float32).reshape(1, D)
    y += out_b[None, None, :]
    return y


# revision 8
# speedup vs baseline: 1.5568x; 1.5568x over previous
"""MultiHeadDiffAttention TRN2 kernel, v2 (pipelined).

Sharding: 8 cores = 2 batches x 4 head-pairs (same as v1). Core c handles
batch c//4 and heads {2g, 2g+1}, g = c%4; its 128 channels form one
GroupNorm group. Final projection is a partial sum over the core's
channels; the host adds 4 partials per batch plus the bias.

v2 structure (vs v1):
  - inputs/weights/outputs in bf16 (halves DMA bytes); attention math in
    fp32r, projections/final matmul in bf16
  - x DMA split into 4 chunks so projections start ~4x earlier
  - ALL psum use during proj+attention goes through two pools that coexist
    within the 8 banks: sc (2 tiles x 2 banks, double-buffered) and av
    (1 x 4 banks)
  - attention pipelined: per key-block, score MMs -> exp -> (deferred one
    group) attn@V MMs, so PE never waits on the activation engine
  - loops ordered h-outer/attn-inner; the diff-attn combine and GroupNorm
    stats for head-pair h run on DVE/Pool while h+1's attention occupies
    PE/ACT
  - exp groups are [128, 1024] (2 key-blocks x 512 queries... actually
    1 key-block x 2 query-chunks, keeping the stationary operand resident
    across consecutive matmuls)
"""

import sys

sys.path.insert(0, "/opt/trn_rl_repo")

import numpy as np
from collections import deque

import concourse.bacc as bacc
import concourse.mybir as mybir
import concourse.tile as tile
from concourse.masks import make_identity
from concourse.bass_utils import run_bass_kernel_spmd

B, S, D = 2, 2048, 512
H = 8
HD = D // H          # 64
CH = 2 * HD          # 128 channels per core (one GroupNorm group)
LAMBDA_INIT = 0.2
EPS = 1e-5
N_CORES = 8

QB = 512             # query chunk (psum bank)
NQB = S // QB        # 4
KB = 128             # key block
NKB = S // KB        # 16
SB = 128             # seq block for final matmul
NSB = S // SB        # 16

F32 = mybir.dt.float32
F32R = mybir.dt.float32r
BF16 = mybir.dt.bfloat16
NWEIGHTS = 5
WIDX = {"q1": 0, "k1": 1, "q2": 2, "k2": 3, "v": 4}

_CACHE = {}

from contextlib import nullcontext


def build_program(repeats=1, hw_loop=False):
    nc = bacc.Bacc("TRN2", target_bir_lowering=False, debug=False)

    # ---- external I/O (packed per-partition-contiguous host layouts) ----
    # xp[p, c*S + s] = x[b, s, 128c+p]              (bf16)
    d_xp = nc.declare_dram_parameter("xp", [128, 4 * S], BF16, isOutput=False)
    # wp[p, w*512 + c*128 + m] = W_w[ch0+m, 128c+p]; then owT[p, d] tail
    d_wp = nc.declare_dram_parameter("wp", [128, NWEIGHTS * 512 + D],
                                     BF16, isOutput=False)
    # cp[p, :] = [k1b, k2b, gnw, gnb, neglam0, neglam1]
    d_cp = nc.declare_dram_parameter("cp", [CH, 6], F32, isOutput=False)
    # yp[p, sb*D + d] = y_part[128*sb+p, d]         (bf16)
    d_y = nc.declare_dram_parameter("y_part", [SB, NSB * D], BF16,
                                    isOutput=True)
    # yb[0, d] = (gn_b_eff . owT)[d] — constant row added host-side
    d_yb = nc.declare_dram_parameter("yb", [1, D], F32, isOutput=True)

    with tile.TileContext(nc) as tc:
     with (tc.For_i(0, repeats) if hw_loop else nullcontext()):
      for _rep in range(1 if hw_loop else repeats):
        with (
            tc.tile_pool(name="consts", bufs=1) as consts,
            tc.tile_pool(name="qk", bufs=1) as qk_pool,
            tc.tile_pool(name="vaug", bufs=1) as vaug_pool,
            tc.tile_pool(name="xtp", bufs=1) as xt_pool,
            tc.tile_pool(name="upool", bufs=13) as u_pool,
            tc.tile_pool(name="ubig", bufs=1) as ubig_pool,
            tc.tile_pool(name="opool", bufs=1) as o_pool,
            tc.tile_pool(name="small", bufs=1) as small,
        ):
            # ---- constants / packed inputs ----
            ones = consts.tile([128, 1], F32, tag="ones")
            nc.vector.memset(ones, 1.0)
            eps_t = consts.tile([1, 1], F32, tag="eps")
            nc.vector.memset(eps_t, EPS)
            cp = consts.tile([CH, 6], F32, tag="cp")
            nc.sync.dma_start(out=cp, in_=d_cp.ap())
            k1b, k2b = cp[:, 0:1], cp[:, 1:2]
            gnw, gnb = cp[:, 2:3], cp[:, 3:4]
            neglam = cp[:, 4:6]

            wt = consts.tile([128, NWEIGHTS, 4, CH], BF16, tag="wt")
            nc.sync.dma_start(
                out=wt,
                in_=d_wp.ap()[:, 0:NWEIGHTS * 512].rearrange(
                    "p (w c m) -> p w c m", w=NWEIGHTS, c=4))

            xt = xt_pool.tile([128, 4, S], BF16, tag="xt")
            for c in range(4):
                nc.sync.dma_start(out=xt[:, c, :],
                                  in_=d_xp.ap()[:, c * S:(c + 1) * S])

            owT = consts.tile([CH, D], BF16, tag="owT")
            nc.sync.dma_start(out=owT, in_=d_wp.ap()[:, NWEIGHTS * 512:])

            identf = consts.tile([SB, SB], F32, tag="identf")
            make_identity(nc, identf)
            ident = consts.tile([SB, SB], BF16, tag="ident")
            nc.vector.tensor_copy(ident, identf)

            # persistent SBUF tensors
            qk = {w: qk_pool.tile([CH, S], BF16, tag=w, name=w)
                  for w in ("q1", "k1", "q2", "k2")}
            vT = qk_pool.tile([CH, S], BF16, tag="vT")
            va = vaug_pool.tile([SB, 2 * NSB, HD + 1], BF16, tag="va")
            nc.vector.tensor_copy(va[:, :, HD:HD + 1],
                                  ones.to_broadcast((SB, 2 * NSB, 1)))
            oT = o_pool.tile([CH, S], F32, tag="oT")
            xnr = o_pool.tile([CH, S], BF16, tag="xnr")
            U = {(h, a): ubig_pool.tile([HD + 1, S], F32, tag=f"U{h}{a}",
                                         name=f"U{h}{a}")
                 for h in (0, 1) for a in (1, 2)}
            nst = 4
            BST_F = S // nst
            bstats = small.tile([CH, nst, 6], F32, tag="bstats")

            with (
                tc.tile_pool(name="sc", bufs=3, space="PSUM") as sc_pool,
                tc.tile_pool(name="avp", bufs=1, space="PSUM") as av_pool,
            ):
                # ---- projections & v-transpose as small chunks (PE
                # fillers interleaved into the first attention block) ----
                def proj_chunk(w, dst, qb, bias=None):
                    ps = sc_pool.tile([128, QB], F32, tag="sc",
                                      name=f"pj_{w}{qb}")
                    for c in range(4):
                        nc.tensor.matmul(
                            ps, wt[:, WIDX[w], c, :],
                            xt[:, c, qb * QB:(qb + 1) * QB],
                            start=(c == 0), stop=(c == 3))
                    sl = slice(qb * QB, (qb + 1) * QB)
                    if bias is not None:
                        nc.vector.tensor_scalar_add(dst[:, sl], ps, bias)
                    else:
                        nc.vector.tensor_copy(dst[:, sl], ps)

                def vtrans_chunk(grp):
                    pst = sc_pool.tile([128, 4, SB], BF16, tag="sc",
                                       name=f"pv{grp}")
                    for i in range(4):
                        sb = 4 * grp + i
                        nc.tensor.transpose(
                            pst[:, i, :], vT[:, sb * SB:(sb + 1) * SB], ident)
                    nc.vector.tensor_copy(
                        va[:, 8 * grp:8 * grp + 8, 0:HD],
                        pst.rearrange("p i (h m) -> p (i h) m", h=2))

                # ---- attention for one (h, attn): pipelined sc->exp->av,
                # processed in two query-pair chunks so the combine for a
                # chunk overlaps the next chunk's attention ----
                def attention(h, attn, fillers=None, fill_plan=None,
                              chunks=((0, 2 * QB), (2 * QB, 2 * QB))):
                    hs = slice(h * HD, (h + 1) * HD)
                    qT, kT = qk[f"q{attn}"], qk[f"k{attn}"]
                    for pair, (lo, w) in enumerate(chunks):
                        pc = slice(lo, lo + w)
                        av = av_pool.tile([HD + 1, w], F32, tag="av",
                                          name="av")
                        pending = []

                        nsub = w // QB

                        def emit_av(p, av=av, nsub=nsub):
                            ut, kb = p
                            for j in range(nsub):
                                nc.tensor.matmul(
                                    av[:, j * QB:(j + 1) * QB],
                                    va[:, 2 * kb + h, :],
                                    ut[:, j * QB:(j + 1) * QB],
                                    start=(kb == 0), stop=(kb == NKB - 1),
                                )

                        for kb in range(NKB):
                            sct = sc_pool.tile([128, w], F32, tag="sc",
                                               name="sc")
                            for j in range(nsub):
                                q0 = lo + j * QB
                                nc.tensor.matmul(
                                    sct[:, j * QB:(j + 1) * QB],
                                    kT[hs, kb * KB:(kb + 1) * KB],
                                    qT[hs, q0:q0 + QB],
                                    start=True, stop=True,
                                )
                            ut = u_pool.tile([128, w], BF16, tag="u",
                                             name="u")
                            nc.scalar.activation(
                                out=ut, in_=sct,
                                func=mybir.ActivationFunctionType.Exp,
                                scale=1.0 / (HD ** 0.5),
                            )
                            pending.append((ut, kb))
                            if fill_plan is not None:
                                nfill, nflush = fill_plan(pair, kb)
                                for _ in range(nfill):
                                    if fillers:
                                        fillers.popleft()()
                                for _ in range(nflush):
                                    if len(pending) > 1:
                                        emit_av(pending.pop(0))
                            else:
                                if len(pending) > 1:
                                    emit_av(pending.pop(0))
                        for p in pending:
                            emit_av(p)
                        nc.vector.tensor_copy(U[(h, attn)][:, pc], av)
                        if attn == 1:
                            combineA(h, lo, w)
                        else:
                            combineB(h, lo, w)

                t1s = {0: small.tile([HD, S], F32, tag="t1full",
                                     name="t1full")}

                def combineA(h, lo, w):
                    pc = slice(lo, lo + w)
                    rr = small.tile([1, w], F32, tag="rr1", name="rr")
                    nc.vector.reciprocal(out=rr, in_=U[(h, 1)][HD:HD + 1, pc])
                    rb1 = small.tile([HD, w], F32, tag="rb1", name="rb")
                    nc.gpsimd.partition_broadcast(rb1, rr)
                    nc.vector.tensor_mul(t1s[0][:, pc], U[(h, 1)][0:HD, pc],
                                         rb1)

                def combineB(h, lo, w):
                    hs = slice(h * HD, (h + 1) * HD)
                    pc = slice(lo, lo + w)
                    rr = small.tile([1, w], F32, tag="rr2", name="rr")
                    nc.vector.reciprocal(out=rr, in_=U[(h, 2)][HD:HD + 1, pc])
                    rb2 = small.tile([HD, w], F32, tag="rb2", name="rb")
                    nc.gpsimd.partition_broadcast(rb2, rr)
                    t2 = small.tile([HD, w], F32, tag="t2", name="t2")
                    nc.vector.scalar_tensor_tensor(
                        out=t2, in0=U[(h, 2)][0:HD, pc],
                        scalar=neglam[0:HD, h:h + 1], in1=rb2,
                        op0=mybir.AluOpType.mult,
                        op1=mybir.AluOpType.mult,
                    )
                    nc.vector.tensor_add(oT[hs, pc], t1s[0][:, pc], t2)
                    for ii in range(lo // BST_F, (lo + w) // BST_F):
                        nc.vector.bn_stats(
                            out=bstats[hs, ii, :],
                            in_=oT[hs, ii * BST_F:(ii + 1) * BST_F])
                    nc.vector.tensor_copy(xnr[hs, pc], oT[hs, pc])

                # ---- schedule: a 3-chunk prologue starts the exp
                # stream ~3us in; every other projection chunk and the
                # v-transpose ride inside attention(0,1) as PE fillers;
                # attn@V for early key-blocks defers until transposed V
                # chunks exist. ----
                proj_chunk("k1", qk["k1"], 0, k1b)
                proj_chunk("q1", qk["q1"], 0)
                proj_chunk("q1", qk["q1"], 1)
                fillers = deque()
                for qb in (1, 2, 3):
                    fillers.append(lambda qb=qb: proj_chunk("k1", qk["k1"],
                                                            qb, k1b))
                for qb in range(4):
                    fillers.append(lambda qb=qb: proj_chunk("v", vT, qb))
                for g in range(4):
                    fillers.append(lambda g=g: vtrans_chunk(g))
                for qb in (2, 3):
                    fillers.append(lambda qb=qb: proj_chunk("q1", qk["q1"],
                                                            qb))
                for qb in range(4):
                    fillers.append(lambda qb=qb: proj_chunk("q2", qk["q2"],
                                                            qb))
                for qb in range(4):
                    fillers.append(lambda qb=qb: proj_chunk("k2", qk["k2"],
                                                            qb, k2b))

                def fill_plan(pair, kb):
                    it = pair * NKB + kb
                    if it <= 10:
                        return (1, 0)      # 1 filler/iter keeps ACT fed
                    if it <= 16:
                        return (1, 3)      # flush deferred attn@V
                    if it <= 20:
                        return (1, 2)
                    return (0, 2)

                attention(0, 1, fillers, fill_plan)
                attention(0, 2)
                attention(1, 1)
                attention(1, 2, chunks=((0, 2 * QB), (2 * QB, QB),
                                        (3 * QB, QB)))
                # preload the Sqrt activation table while DVE combines
                dummy = small.tile([1, 1], F32, tag="dummy")
                nc.scalar.activation(out=dummy, in_=eps_t,
                                     func=mybir.ActivationFunctionType.Sqrt,
                                     scale=1.0)

            # ---- GroupNorm global stats ----
            with tc.tile_pool(name="stp", bufs=1, space="PSUM") as stp_pool:
                mv = small.tile([CH, 2], F32, tag="mv")
                nc.vector.bn_aggr(out=mv, in_=bstats)
                s12 = small.tile([CH, 2], F32, tag="s12")
                nc.vector.tensor_copy(s12[:, 0:1], mv[:, 0:1])
                nc.vector.scalar_tensor_tensor(
                    out=s12[:, 1:2], in0=mv[:, 0:1], scalar=0.0,
                    in1=mv[:, 0:1], op0=mybir.AluOpType.add,
                    op1=mybir.AluOpType.mult)
                nc.vector.tensor_add(s12[:, 1:2], s12[:, 1:2], mv[:, 1:2])
                st = stp_pool.tile([1, 2], F32, tag="st")
                nc.tensor.matmul(st[0:1, 0:1], s12[:, 0:1], ones,
                                 start=True, stop=True)
                nc.tensor.matmul(st[0:1, 1:2], s12[:, 1:2], ones,
                                 start=True, stop=True, skip_group_check=True)
                mu_e2 = small.tile([1, 2], F32, tag="mu_e2")
                nc.vector.tensor_scalar_mul(mu_e2, st[0:1, 0:2], 1.0 / CH)
                sqm = small.tile([1, 1], F32, tag="sqm")
                nc.vector.tensor_mul(sqm, mu_e2[:, 0:1], mu_e2[:, 0:1])
                var = small.tile([1, 1], F32, tag="var")
                nc.vector.tensor_sub(var, mu_e2[:, 1:2], sqm)
                std = small.tile([1, 1], F32, tag="std")
                nc.scalar.activation(out=std, in_=var,
                                     func=mybir.ActivationFunctionType.Sqrt,
                                     bias=eps_t, scale=1.0)
                rstd = small.tile([1, 1], F32, tag="rstd")
                nc.vector.reciprocal(out=rstd, in_=std)
                murstd = small.tile([1, 2], F32, tag="murstd")
                nc.vector.tensor_copy(murstd[:, 0:1], mu_e2[:, 0:1])
                nc.vector.tensor_copy(murstd[:, 1:2], rstd)
                br = small.tile([CH, 2], F32, tag="br")
                nc.gpsimd.partition_broadcast(br, murstd)
                a_t = small.tile([CH, 1], F32, tag="a_t")
                nc.vector.tensor_mul(a_t, br[:, 1:2], gnw)
                amu = small.tile([CH, 1], F32, tag="amu")
                nc.vector.tensor_mul(amu, a_t, br[:, 0:1])
                b_t = small.tile([CH, 1], F32, tag="b_t")
                nc.vector.tensor_sub(b_t, gnb, amu)
                # fold GN affine into the output projection:
                #   y = xnr.T @ (a*owT) + (b.T @ owT)
                owTs = small.tile([CH, D], BF16, tag="owTs")
                nc.vector.tensor_scalar_mul(owTs, owT, a_t)
                b16 = small.tile([CH, 1], BF16, tag="b16")
                nc.vector.tensor_copy(b16, b_t)
                ybp = stp_pool.tile([1, D], F32, tag="ybp")
                nc.tensor.matmul(ybp, b16, owT, start=True, stop=True,
                                 skip_group_check=True)
                yb = small.tile([1, D], F32, tag="yb")
                nc.vector.tensor_copy(yb, ybp)
                nc.sync.dma_start(out=d_yb.ap(), in_=yb)

            # ---- final projection partial: y = xnr.T @ owTs ----
            with (
                tc.tile_pool(name="fin", bufs=2, space="PSUM") as fin_pool,
                tc.tile_pool(name="ytp", bufs=2) as yt_pool,
            ):
                half = NSB // 4
                for hf in range(4):
                    ps = fin_pool.tile([SB, half * D], F32, tag="fin",
                                       name="fin")
                    yt = yt_pool.tile([SB, half, D], BF16, tag="yt", name="yt")
                    for i in range(half):
                        sb = hf * half + i
                        nc.tensor.matmul(
                            ps[:, i * D:(i + 1) * D],
                            xnr[:, sb * SB:(sb + 1) * SB],
                            owTs,
                            start=True, stop=True,
                        )
                    nc.scalar.activation(
                        out=yt, in_=ps.rearrange("p (i d) -> p i d", i=half),
                        func=mybir.ActivationFunctionType.Copy, scale=1.0)
                    nc.sync.dma_start(
                        out=d_y.ap().rearrange(
                            "p (hf sb d) -> p hf sb d", hf=4, sb=half)[:, hf],
                        in_=yt)

    nc.compile()
    return nc


def _shard_inputs(inputs):
    import ml_dtypes
    bf = ml_dtypes.bfloat16
    x = np.ascontiguousarray(inputs["x"], np.float32)
    lam = (np.exp(inputs["lambda_q1"] * inputs["lambda_k1"])
           - np.exp(inputs["lambda_q2"] * inputs["lambda_k2"])
           + LAMBDA_INIT).astype(np.float32).reshape(H)
    in_maps = []
    for c in range(N_CORES):
        b, g = divmod(c, 4)
        ch = slice(CH * g, CH * (g + 1))
        # xp[p, c*S+s] = x[b, s, 128c+p]
        xp = np.ascontiguousarray(
            x[b].T.reshape(4, 128, S).transpose(1, 0, 2).reshape(128, 4 * S)
        ).astype(bf)
        wlist = []
        for W in (inputs["Q1_w"], inputs["K1_w"], inputs["Q2_w"],
                  inputs["K2_w"], inputs["V_w"]):
            wT = np.asarray(W)[ch].T  # [512, 128]
            wlist.append(np.ascontiguousarray(
                wT.reshape(4, 128, CH).transpose(1, 0, 2).reshape(128, 512)))
        owT = np.ascontiguousarray(np.asarray(inputs["out_w"])[:, ch].T)
        wp = np.concatenate(wlist + [owT], axis=1).astype(bf)
        cp = np.stack([
            np.asarray(inputs["K1_b"])[ch],
            np.asarray(inputs["K2_b"])[ch],
            np.asarray(inputs["gn_w"])[ch],
            np.asarray(inputs["gn_b"])[ch],
            np.full(CH, -lam[2 * g], np.float32),
            np.full(CH, -lam[2 * g + 1], np.float32),
        ], axis=1).astype(np.float32)
        in_maps.append({"xp": xp, "wp": wp, "cp": np.ascontiguousarray(cp)})
    return in_maps


def kernel(**inputs):
    inputs = {k: np.asarray(v) for k, v in inputs.items()}
    if "nc" not in _CACHE:
        _CACHE["nc"] = build_program()
    nc = _CACHE["nc"]
    in_maps = _shard_inputs(inputs)
    res = run_bass_kernel_spmd(nc, in_maps, list(range(N_CORES)))
    out_b = np.asarray(inputs["out_b"], np.float32)
    y = np.zeros((B, S, D), np.float32)
    for c in range(N_CORES):
        b = c // 4
        yp = res.results[c]["y_part"].astype(np.float32)
        y[b] += yp.reshape(SB, NSB, D).transpose(1, 0, 2).reshape(S, D)
        y[b] += res.results[c]["yb"].astype(np.float32).reshape(1, D)
    y += out_b[None, None, :]
    return y


# revision 10
# speedup vs baseline: 1.9034x; 1.2227x over previous
"""MultiHeadDiffAttention TRN2 kernel (pipelined, bf16).

Sharding: 8 cores = 2 batches x 4 head-pairs. Core c handles batch c//4 and
heads {2g, 2g+1}, g = c%4; its 128 channels form one GroupNorm group. The
final projection is a partial sum over the core's channels; the host adds
the 4 partials per batch, a per-core bias row (yb), and the output bias.

Design (evolved v1 -> v8 against CoreSim cost-model profiles and HW A/B
probes; HW body time 603us -> ~170-240us depending on terminal load):
  - all matmul operands bf16 (inputs/weights DMA'd bf16; q/k/v/scores path
    bf16). fp32r attention was the original HW bottleneck: fp32r is
    excluded from fast-weight-load, making every 213ns matmul cost ~580ns.
    bf16 keeps rel err ~6.4e-3 (tolerance 2e-2).
  - attention pipelined per (head, attn, 1024-query chunk): per key block,
    2 score MMs (one stationary load) -> one [128,1024] exp on ACT ->
    attn@V MMs deferred one group so PE never waits on ACT. Score psum is
    triple-buffered (2 banks each), attn@V accumulates in a 2-bank tile
    (psum exactly 8 banks). The ones-column appended to V yields the
    softmax denominator for free.
  - every projection chunk (512 cols x 4 contract blocks) and the
    PE-transpose of V ride inside the first attention block as PE fillers
    (1/iteration), so the exp stream starts ~3us in and ACT stays fed.
  - the diff-attn combine (U1/d1 - lam*U2/d2) and GroupNorm bn_stats for a
    chunk run on DVE/Pool under the next chunk's attention; the last
    attention call narrows to 512-col chunks to shrink the exposed tail.
  - GroupNorm's affine is folded into the output projection
    (y = oT^T @ (a*owT) + b^T owT), so only raw bf16 copies of oT are on
    the critical tail; final psum->bf16 copies go through ACT (idle then),
    and the Sqrt activation table is preloaded under the combine.

Timing note: measure with build_program(repeats=N, hw_loop=True) (For_i
device loop) and the slope between two trip counts; host dispatch is
~75-90ms/call and terminal load drifts +/-20%, so only within-process
comparisons are meaningful.
"""

import sys

sys.path.insert(0, "/opt/trn_rl_repo")

import numpy as np
from collections import deque

import concourse.bacc as bacc
import concourse.mybir as mybir
import concourse.tile as tile
from concourse.masks import make_identity
from concourse.bass_utils import run_bass_kernel_spmd

B, S, D = 2, 2048, 512
H = 8
HD = D // H          # 64
CH = 2 * HD          # 128 channels per core (one GroupNorm group)
LAMBDA_INIT = 0.2
EPS = 1e-5
N_CORES = 8

QB = 512             # query chunk (psum bank)
NQB = S // QB        # 4
KB = 128             # key block
NKB = S // KB        # 16
SB = 128             # seq block for final matmul
NSB = S // SB        # 16

F32 = mybir.dt.float32
F32R = mybir.dt.float32r
BF16 = mybir.dt.bfloat16
NWEIGHTS = 5
WIDX = {"q1": 0, "k1": 1, "q2": 2, "k2": 3, "v": 4}

_CACHE = {}

from contextlib import nullcontext


def build_program(repeats=1, hw_loop=False):
    nc = bacc.Bacc("TRN2", target_bir_lowering=False, debug=False)

    # ---- external I/O (packed per-partition-contiguous host layouts) ----
    # xp[p, c*S + s] = x[b, s, 128c+p]              (bf16)
    d_xp = nc.declare_dram_parameter("xp", [128, 4 * S], BF16, isOutput=False)
    # wp[p, w*512 + c*128 + m] = W_w[ch0+m, 128c+p]; then owT[p, d] tail
    d_wp = nc.declare_dram_parameter("wp", [128, NWEIGHTS * 512 + D],
                                     BF16, isOutput=False)
    # cp[p, :] = [k1b, k2b, gnw, gnb, neglam0, neglam1]
    d_cp = nc.declare_dram_parameter("cp", [CH, 6], F32, isOutput=False)
    # yp[p, sb*D + d] = y_part[128*sb+p, d]         (bf16)
    d_y = nc.declare_dram_parameter("y_part", [SB, NSB * D], BF16,
                                    isOutput=True)
    # yb[0, d] = (gn_b_eff . owT)[d] — constant row added host-side
    d_yb = nc.declare_dram_parameter("yb", [1, D], F32, isOutput=True)

    with tile.TileContext(nc) as tc:
     with (tc.For_i(0, repeats) if hw_loop else nullcontext()):
      for _rep in range(1 if hw_loop else repeats):
        with (
            tc.tile_pool(name="consts", bufs=1) as consts,
            tc.tile_pool(name="qk", bufs=1) as qk_pool,
            tc.tile_pool(name="vaug", bufs=1) as vaug_pool,
            tc.tile_pool(name="xtp", bufs=1) as xt_pool,
            tc.tile_pool(name="upool", bufs=13) as u_pool,
            tc.tile_pool(name="ubig", bufs=1) as ubig_pool,
            tc.tile_pool(name="opool", bufs=1) as o_pool,
            tc.tile_pool(name="small", bufs=1) as small,
        ):
            # ---- constants / packed inputs ----
            ones = consts.tile([128, 1], F32, tag="ones")
            nc.vector.memset(ones, 1.0)
            eps_t = consts.tile([1, 1], F32, tag="eps")
            nc.vector.memset(eps_t, EPS)
            cp = consts.tile([CH, 6], F32, tag="cp")
            nc.sync.dma_start(out=cp, in_=d_cp.ap())
            k1b, k2b = cp[:, 0:1], cp[:, 1:2]
            gnw, gnb = cp[:, 2:3], cp[:, 3:4]
            neglam = cp[:, 4:6]

            wt = consts.tile([128, NWEIGHTS, 4, CH], BF16, tag="wt")
            nc.sync.dma_start(
                out=wt,
                in_=d_wp.ap()[:, 0:NWEIGHTS * 512].rearrange(
                    "p (w c m) -> p w c m", w=NWEIGHTS, c=4))

            xt = xt_pool.tile([128, 4, S], BF16, tag="xt")
            for half in range(2):
                hsl = slice(half * (S // 2), (half + 1) * (S // 2))
                for c in range(4):
                    nc.sync.dma_start(
                        out=xt[:, c, hsl],
                        in_=d_xp.ap()[:, c * S + half * (S // 2):
                                      c * S + (half + 1) * (S // 2)])

            owT = consts.tile([CH, D], BF16, tag="owT")
            nc.sync.dma_start(out=owT, in_=d_wp.ap()[:, NWEIGHTS * 512:])

            identf = consts.tile([SB, SB], F32, tag="identf")
            make_identity(nc, identf)
            ident = consts.tile([SB, SB], BF16, tag="ident")
            nc.vector.tensor_copy(ident, identf)

            # persistent SBUF tensors
            qk = {w: qk_pool.tile([CH, S], BF16, tag=w, name=w)
                  for w in ("q1", "k1", "q2", "k2")}
            vT = qk_pool.tile([CH, S], BF16, tag="vT")
            va = vaug_pool.tile([SB, 2 * NSB, HD + 1], BF16, tag="va")
            nc.vector.tensor_copy(va[:, :, HD:HD + 1],
                                  ones.to_broadcast((SB, 2 * NSB, 1)))
            oT = o_pool.tile([CH, S], F32, tag="oT")
            xnr = o_pool.tile([CH, S], BF16, tag="xnr")
            U = {(h, a): ubig_pool.tile([HD + 1, S], F32, tag=f"U{h}{a}",
                                         name=f"U{h}{a}")
                 for h in (0, 1) for a in (1, 2)}
            nst = 4
            BST_F = S // nst
            bstats = small.tile([CH, nst, 6], F32, tag="bstats")

            with (
                tc.tile_pool(name="sc", bufs=3, space="PSUM") as sc_pool,
                tc.tile_pool(name="avp", bufs=1, space="PSUM") as av_pool,
            ):
                # ---- projections & v-transpose as small chunks (PE
                # fillers interleaved into the first attention block) ----
                def proj_chunk(w, dst, qb, bias=None):
                    ps = sc_pool.tile([128, QB], F32, tag="sc",
                                      name=f"pj_{w}{qb}")
                    for c in range(4):
                        nc.tensor.matmul(
                            ps, wt[:, WIDX[w], c, :],
                            xt[:, c, qb * QB:(qb + 1) * QB],
                            start=(c == 0), stop=(c == 3))
                    sl = slice(qb * QB, (qb + 1) * QB)
                    if bias is not None:
                        nc.vector.tensor_scalar_add(dst[:, sl], ps, bias)
                    else:
                        nc.vector.tensor_copy(dst[:, sl], ps)

                def vtrans_chunk(grp):
                    pst = sc_pool.tile([128, 4, SB], BF16, tag="sc",
                                       name=f"pv{grp}")
                    for i in range(4):
                        sb = 4 * grp + i
                        nc.tensor.transpose(
                            pst[:, i, :], vT[:, sb * SB:(sb + 1) * SB], ident)
                    nc.vector.tensor_copy(
                        va[:, 8 * grp:8 * grp + 8, 0:HD],
                        pst.rearrange("p i (h m) -> p (i h) m", h=2))

                # ---- attention for one (h, attn): pipelined sc->exp->av,
                # processed in two query-pair chunks so the combine for a
                # chunk overlaps the next chunk's attention ----
                def attention(h, attn, fillers=None, fill_plan=None,
                              chunks=((0, 2 * QB), (2 * QB, 2 * QB))):
                    hs = slice(h * HD, (h + 1) * HD)
                    qT, kT = qk[f"q{attn}"], qk[f"k{attn}"]
                    for pair, (lo, w) in enumerate(chunks):
                        pc = slice(lo, lo + w)
                        av = av_pool.tile([HD + 1, w], F32, tag="av",
                                          name="av")
                        pending = []

                        nsub = w // QB

                        def emit_av(p, av=av, nsub=nsub):
                            ut, kb = p
                            for j in range(nsub):
                                nc.tensor.matmul(
                                    av[:, j * QB:(j + 1) * QB],
                                    va[:, 2 * kb + h, :],
                                    ut[:, j * QB:(j + 1) * QB],
                                    start=(kb == 0), stop=(kb == NKB - 1),
                                )

                        for kb in range(NKB):
                            sct = sc_pool.tile([128, w], F32, tag="sc",
                                               name="sc")
                            for j in range(nsub):
                                q0 = lo + j * QB
                                nc.tensor.matmul(
                                    sct[:, j * QB:(j + 1) * QB],
                                    kT[hs, kb * KB:(kb + 1) * KB],
                                    qT[hs, q0:q0 + QB],
                                    start=True, stop=True,
                                )
                            ut = u_pool.tile([128, w], BF16, tag="u",
                                             name="u")
                            nc.scalar.activation(
                                out=ut, in_=sct,
                                func=mybir.ActivationFunctionType.Exp,
                                scale=1.0 / (HD ** 0.5),
                            )
                            pending.append((ut, kb))
                            if fill_plan is not None:
                                nfill, nflush = fill_plan(pair, kb)
                                for _ in range(nfill):
                                    if fillers:
                                        fillers.popleft()()
                                for _ in range(nflush):
                                    if len(pending) > 1:
                                        emit_av(pending.pop(0))
                            else:
                                if len(pending) > 1:
                                    emit_av(pending.pop(0))
                        for p in pending:
                            emit_av(p)
                        nc.vector.tensor_copy(U[(h, attn)][:, pc], av)
                        if attn == 1:
                            combineA(h, lo, w)
                        else:
                            combineB(h, lo, w)

                t1s = {0: small.tile([HD, S], F32, tag="t1full",
                                     name="t1full")}

                def combineA(h, lo, w):
                    pc = slice(lo, lo + w)
                    rr = small.tile([1, w], F32, tag="rr1", name="rr")
                    nc.vector.reciprocal(out=rr, in_=U[(h, 1)][HD:HD + 1, pc])
                    rb1 = small.tile([HD, w], F32, tag="rb1", name="rb")
                    nc.gpsimd.partition_broadcast(rb1, rr)
                    nc.vector.tensor_mul(t1s[0][:, pc], U[(h, 1)][0:HD, pc],
                                         rb1)

                def combineB(h, lo, w):
                    hs = slice(h * HD, (h + 1) * HD)
                    pc = slice(lo, lo + w)
                    rr = small.tile([1, w], F32, tag="rr2", name="rr")
                    nc.vector.reciprocal(out=rr, in_=U[(h, 2)][HD:HD + 1, pc])
                    rb2 = small.tile([HD, w], F32, tag="rb2", name="rb")
                    nc.gpsimd.partition_broadcast(rb2, rr)
                    t2 = small.tile([HD, w], F32, tag="t2", name="t2")
                    nc.vector.scalar_tensor_tensor(
                        out=t2, in0=U[(h, 2)][0:HD, pc],
                        scalar=neglam[0:HD, h:h + 1], in1=rb2,
                        op0=mybir.AluOpType.mult,
                        op1=mybir.AluOpType.mult,
                    )
                    nc.vector.tensor_add(oT[hs, pc], t1s[0][:, pc], t2)
                    for ii in range(lo // BST_F, (lo + w) // BST_F):
                        nc.vector.bn_stats(
                            out=bstats[hs, ii, :],
                            in_=oT[hs, ii * BST_F:(ii + 1) * BST_F])
                    nc.vector.tensor_copy(xnr[hs, pc], oT[hs, pc])

                # ---- schedule: a 3-chunk prologue starts the exp
                # stream ~3us in; every other projection chunk and the
                # v-transpose ride inside attention(0,1) as PE fillers;
                # attn@V for early key-blocks defers until transposed V
                # chunks exist. ----
                proj_chunk("k1", qk["k1"], 0, k1b)
                proj_chunk("q1", qk["q1"], 0)
                proj_chunk("q1", qk["q1"], 1)
                fillers = deque()
                for qb in (1, 2, 3):
                    fillers.append(lambda qb=qb: proj_chunk("k1", qk["k1"],
                                                            qb, k1b))
                for qb in range(4):
                    fillers.append(lambda qb=qb: proj_chunk("v", vT, qb))
                for g in range(4):
                    fillers.append(lambda g=g: vtrans_chunk(g))
                for qb in (2, 3):
                    fillers.append(lambda qb=qb: proj_chunk("q1", qk["q1"],
                                                            qb))
                for qb in range(4):
                    fillers.append(lambda qb=qb: proj_chunk("q2", qk["q2"],
                                                            qb))
                for qb in range(4):
                    fillers.append(lambda qb=qb: proj_chunk("k2", qk["k2"],
                                                            qb, k2b))

                def fill_plan(pair, kb):
                    it = pair * NKB + kb
                    if it <= 10:
                        return (1, 0)      # 1 filler/iter keeps ACT fed
                    if it <= 16:
                        return (1, 3)      # flush deferred attn@V
                    if it <= 20:
                        return (1, 2)
                    return (0, 2)

                attention(0, 1, fillers, fill_plan)
                attention(0, 2)
                attention(1, 1)
                attention(1, 2, chunks=((0, 2 * QB), (2 * QB, QB),
                                        (3 * QB, QB)))
                # preload the Sqrt activation table while DVE combines
                dummy = small.tile([1, 1], F32, tag="dummy")
                nc.scalar.activation(out=dummy, in_=eps_t,
                                     func=mybir.ActivationFunctionType.Sqrt,
                                     scale=1.0)

            # ---- GroupNorm global stats ----
            with tc.tile_pool(name="stp", bufs=1, space="PSUM") as stp_pool:
                mv = small.tile([CH, 2], F32, tag="mv")
                nc.vector.bn_aggr(out=mv, in_=bstats)
                s12 = small.tile([CH, 2], F32, tag="s12")
                nc.vector.tensor_copy(s12[:, 0:1], mv[:, 0:1])
                nc.vector.scalar_tensor_tensor(
                    out=s12[:, 1:2], in0=mv[:, 0:1], scalar=0.0,
                    in1=mv[:, 0:1], op0=mybir.AluOpType.add,
                    op1=mybir.AluOpType.mult)
                nc.vector.tensor_add(s12[:, 1:2], s12[:, 1:2], mv[:, 1:2])
                st = stp_pool.tile([1, 2], F32, tag="st")
                nc.tensor.matmul(st[0:1, 0:1], s12[:, 0:1], ones,
                                 start=True, stop=True)
                nc.tensor.matmul(st[0:1, 1:2], s12[:, 1:2], ones,
                                 start=True, stop=True, skip_group_check=True)
                mu_e2 = small.tile([1, 2], F32, tag="mu_e2")
                nc.vector.tensor_scalar_mul(mu_e2, st[0:1, 0:2], 1.0 / CH)
                sqm = small.tile([1, 1], F32, tag="sqm")
                nc.vector.tensor_mul(sqm, mu_e2[:, 0:1], mu_e2[:, 0:1])
                var = small.tile([1, 1], F32, tag="var")
                nc.vector.tensor_sub(var, mu_e2[:, 1:2], sqm)
                std = small.tile([1, 1], F32, tag="std")
                nc.scalar.activation(out=std, in_=var,
                                     func=mybir.ActivationFunctionType.Sqrt,
                                     bias=eps_t, scale=1.0)
                rstd = small.tile([1, 1], F32, tag="rstd")
                nc.vector.reciprocal(out=rstd, in_=std)
                murstd = small.tile([1, 2], F32, tag="murstd")
                nc.vector.tensor_copy(murstd[:, 0:1], mu_e2[:, 0:1])
                nc.vector.tensor_copy(murstd[:, 1:2], rstd)
                br = small.tile([CH, 2], F32, tag="br")
                nc.gpsimd.partition_broadcast(br, murstd)
                a_t = small.tile([CH, 1], F32, tag="a_t")
                nc.vector.tensor_mul(a_t, br[:, 1:2], gnw)
                amu = small.tile([CH, 1], F32, tag="amu")
                nc.vector.tensor_mul(amu, a_t, br[:, 0:1])
                b_t = small.tile([CH, 1], F32, tag="b_t")
                nc.vector.tensor_sub(b_t, gnb, amu)
                # fold GN affine into the output projection:
                #   y = xnr.T @ (a*owT) + (b.T @ owT)
                owTs = small.tile([CH, D], BF16, tag="owTs")
                nc.vector.tensor_scalar_mul(owTs, owT, a_t)
                b16 = small.tile([CH, 1], BF16, tag="b16")
                nc.vector.tensor_copy(b16, b_t)
                ybp = stp_pool.tile([1, D], F32, tag="ybp")
                nc.tensor.matmul(ybp, b16, owT, start=True, stop=True,
                                 skip_group_check=True)
                yb = small.tile([1, D], F32, tag="yb")
                nc.vector.tensor_copy(yb, ybp)
                nc.sync.dma_start(out=d_yb.ap(), in_=yb)

            # ---- final projection partial: y = xnr.T @ owTs ----
            with (
                tc.tile_pool(name="fin", bufs=2, space="PSUM") as fin_pool,
                tc.tile_pool(name="ytp", bufs=2) as yt_pool,
            ):
                half = NSB // 4
                for hf in range(4):
                    ps = fin_pool.tile([SB, half * D], F32, tag="fin",
                                       name="fin")
                    yt = yt_pool.tile([SB, half, D], BF16, tag="yt", name="yt")
                    for i in range(half):
                        sb = hf * half + i
                        nc.tensor.matmul(
                            ps[:, i * D:(i + 1) * D],
                            xnr[:, sb * SB:(sb + 1) * SB],
                            owTs,
                            start=True, stop=True,
                        )
                    if hf % 2 == 0:
                        nc.scalar.activation(
                            out=yt,
                            in_=ps.rearrange("p (i d) -> p i d", i=half),
                            func=mybir.ActivationFunctionType.Copy, scale=1.0)
                    else:
                        nc.vector.tensor_copy(yt, ps.rearrange(
                            "p (i d) -> p i d", i=half))
                    nc.sync.dma_start(
                        out=d_y.ap().rearrange(
                            "p (hf sb d) -> p hf sb d", hf=4, sb=half)[:, hf],
                        in_=yt)

    nc.compile()
    return nc


def _shard_inputs(inputs):
    import ml_dtypes
    bf = ml_dtypes.bfloat16
    x = np.ascontiguousarray(inputs["x"], np.float32)
    lam = (np.exp(inputs["lambda_q1"] * inputs["lambda_k1"])
           - np.exp(inputs["lambda_q2"] * inputs["lambda_k2"])
           + LAMBDA_INIT).astype(np.float32).reshape(H)
    in_maps = []
    for c in range(N_CORES):
        b, g = divmod(c, 4)
        ch = slice(CH * g, CH * (g + 1))
        # xp[p, c*S+s] = x[b, s, 128c+p]
        xp = np.ascontiguousarray(
            x[b].T.reshape(4, 128, S).transpose(1, 0, 2).reshape(128, 4 * S)
        ).astype(bf)
        wlist = []
        for W in (inputs["Q1_w"], inputs["K1_w"], inputs["Q2_w"],
                  inputs["K2_w"], inputs["V_w"]):
            wT = np.asarray(W)[ch].T  # [512, 128]
            wlist.append(np.ascontiguousarray(
                wT.reshape(4, 128, CH).transpose(1, 0, 2).reshape(128, 512)))
        owT = np.ascontiguousarray(np.asarray(inputs["out_w"])[:, ch].T)
        wp = np.concatenate(wlist + [owT], axis=1).astype(bf)
        cp = np.stack([
            np.asarray(inputs["K1_b"])[ch],
            np.asarray(inputs["K2_b"])[ch],
            np.asarray(inputs["gn_w"])[ch],
            np.asarray(inputs["gn_b"])[ch],
            np.full(CH, -lam[2 * g], np.float32),
            np.full(CH, -lam[2 * g + 1], np.float32),
        ], axis=1).astype(np.float32)
        in_maps.append({"xp": xp, "wp": wp, "cp": np.ascontiguousarray(cp)})
    return in_maps


def kernel(**inputs):
    inputs = {k: np.asarray(v) for k, v in inputs.items()}
    if "nc" not in _CACHE:
        _CACHE["nc"] = build_program()
    nc = _CACHE["nc"]
    in_maps = _shard_inputs(inputs)
    res = run_bass_kernel_spmd(nc, in_maps, list(range(N_CORES)))
    out_b = np.asarray(inputs["out_b"], np.float32)
    y = np.zeros((B, S, D), np.float32)
    for c in range(N_CORES):
        b = c // 4
        yp = res.results[c]["y_part"].astype(np.float32)
        y[b] += yp.reshape(SB, NSB, D).transpose(1, 0, 2).reshape(S, D)
        y[b] += res.results[c]["yb"].astype(np.float32).reshape(1, D)
    y += out_b[None, None, :]
    return y


# revision 11
# speedup vs baseline: 1.9576x; 1.0285x over previous
"""MultiHeadDiffAttention TRN2 kernel (pipelined, bf16).

Sharding: 8 cores = 2 batches x 4 head-pairs. Core c handles batch c//4 and
heads {2g, 2g+1}, g = c%4; its 128 channels form one GroupNorm group. The
final projection is a partial sum over the core's channels; the host adds
the 4 partials per batch, a per-core bias row (yb), and the output bias.

Design (evolved v1 -> v8 against CoreSim cost-model profiles and HW A/B
probes; HW body time 603us -> ~165-215us depending on terminal load):
  - all matmul operands bf16 (inputs/weights DMA'd bf16; q/k/v/scores path
    bf16). fp32r attention was the original HW bottleneck: fp32r is
    excluded from fast-weight-load, making every 213ns matmul cost ~580ns.
    bf16 keeps rel err ~6.4e-3 (tolerance 2e-2).
  - attention pipelined per (head, attn, 1024-query chunk): per key block,
    2 score MMs (one stationary load) -> one [128,1024] exp on ACT ->
    attn@V MMs deferred one group so PE never waits on ACT. Score psum is
    triple-buffered (2 banks each), attn@V accumulates in a 2-bank tile
    (psum exactly 8 banks). The ones-column appended to V yields the
    softmax denominator for free.
  - every projection chunk (512 cols x 4 contract blocks) and the
    PE-transpose of V ride inside the first attention block as PE fillers
    (1/iteration), so the exp stream starts ~3us in and ACT stays fed.
  - the diff-attn combine (U1/d1 - lam*U2/d2) and GroupNorm bn_stats for a
    chunk run on DVE/Pool under the next chunk's attention; the last
    attention call narrows to 512-col chunks to shrink the exposed tail.
  - GroupNorm's affine is folded away entirely: gn_w is multiplied into
    the output weights HOST-side, the global 1/sigma is applied at the
    psum->bf16 output copies (per-partition broadcast scalar, alternating
    ACT/DVE), and the tiny partition-sum matmuls borrow score-pool psum —
    so the 16 final matmuls unblock right after the last attention chunk
    instead of waiting for the GroupNorm scalar chain. The Sqrt activation
    table is preloaded under the combine.

Timing note: measure with build_program(repeats=N, hw_loop=True) (For_i
device loop) and the slope between two trip counts; host dispatch is
~75-90ms/call and terminal load drifts +/-20%, so only within-process
comparisons are meaningful.
"""

import sys

sys.path.insert(0, "/opt/trn_rl_repo")

import numpy as np
from collections import deque

import concourse.bacc as bacc
import concourse.mybir as mybir
import concourse.tile as tile
from concourse.masks import make_identity
from concourse.bass_utils import run_bass_kernel_spmd

B, S, D = 2, 2048, 512
H = 8
HD = D // H          # 64
CH = 2 * HD          # 128 channels per core (one GroupNorm group)
LAMBDA_INIT = 0.2
EPS = 1e-5
N_CORES = 8

QB = 512             # query chunk (psum bank)
NQB = S // QB        # 4
KB = 128             # key block
NKB = S // KB        # 16
SB = 128             # seq block for final matmul
NSB = S // SB        # 16

F32 = mybir.dt.float32
F32R = mybir.dt.float32r
BF16 = mybir.dt.bfloat16
NWEIGHTS = 5
WIDX = {"q1": 0, "k1": 1, "q2": 2, "k2": 3, "v": 4}

_CACHE = {}

from contextlib import nullcontext


def build_program(repeats=1, hw_loop=False):
    nc = bacc.Bacc("TRN2", target_bir_lowering=False, debug=False)

    # ---- external I/O (packed per-partition-contiguous host layouts) ----
    # xp[p, c*S + s] = x[b, s, 128c+p]              (bf16)
    d_xp = nc.declare_dram_parameter("xp", [128, 4 * S], BF16, isOutput=False)
    # wp[p, w*512 + c*128 + m] = W_w[ch0+m, 128c+p]; then owT[p, d] tail
    d_wp = nc.declare_dram_parameter("wp", [128, NWEIGHTS * 512 + 2 * D],
                                     BF16, isOutput=False)
    # cp[p, :] = [k1b, k2b, gnw, gnb, neglam0, neglam1]
    d_cp = nc.declare_dram_parameter("cp", [CH, 6], F32, isOutput=False)
    # yp[p, sb*D + d] = y_part[128*sb+p, d]         (bf16)
    d_y = nc.declare_dram_parameter("y_part", [SB, NSB * D], BF16,
                                    isOutput=True)
    # yb[0, d] = (gn_b_eff . owT)[d] — constant row added host-side
    d_yb = nc.declare_dram_parameter("yb", [1, D], F32, isOutput=True)

    with tile.TileContext(nc) as tc:
     with (tc.For_i(0, repeats) if hw_loop else nullcontext()):
      for _rep in range(1 if hw_loop else repeats):
        with (
            tc.tile_pool(name="consts", bufs=1) as consts,
            tc.tile_pool(name="qk", bufs=1) as qk_pool,
            tc.tile_pool(name="vaug", bufs=1) as vaug_pool,
            tc.tile_pool(name="xtp", bufs=1) as xt_pool,
            tc.tile_pool(name="upool", bufs=13) as u_pool,
            tc.tile_pool(name="ubig", bufs=1) as ubig_pool,
            tc.tile_pool(name="opool", bufs=1) as o_pool,
            tc.tile_pool(name="small", bufs=1) as small,
        ):
            # ---- constants / packed inputs ----
            ones = consts.tile([128, 1], F32, tag="ones")
            nc.vector.memset(ones, 1.0)
            eps_t = consts.tile([1, 1], F32, tag="eps")
            nc.vector.memset(eps_t, EPS)
            cp = consts.tile([CH, 6], F32, tag="cp")
            nc.sync.dma_start(out=cp, in_=d_cp.ap())
            k1b, k2b = cp[:, 0:1], cp[:, 1:2]
            gnw, gnb = cp[:, 2:3], cp[:, 3:4]
            neglam = cp[:, 4:6]

            wt = consts.tile([128, NWEIGHTS, 4, CH], BF16, tag="wt")
            # q1+k1 weight slices first: they gate the prologue projections
            nc.sync.dma_start(
                out=wt[:, 0:2],
                in_=d_wp.ap()[:, 0:1024].rearrange(
                    "p (w c m) -> p w c m", w=2, c=4))

            xt = xt_pool.tile([128, 4, S], BF16, tag="xt")
            for half in range(2):
                hsl = slice(half * (S // 2), (half + 1) * (S // 2))
                for c in range(4):
                    nc.sync.dma_start(
                        out=xt[:, c, hsl],
                        in_=d_xp.ap()[:, c * S + half * (S // 2):
                                      c * S + (half + 1) * (S // 2)])
                if half == 0:
                    nc.sync.dma_start(
                        out=wt[:, 2:NWEIGHTS],
                        in_=d_wp.ap()[:, 1024:NWEIGHTS * 512].rearrange(
                            "p (w c m) -> p w c m", w=3, c=4))

            # owT carries gn_w pre-folded (host); owT2 is unscaled (yb row)
            owT = consts.tile([CH, D], BF16, tag="owT")
            nc.sync.dma_start(
                out=owT, in_=d_wp.ap()[:, NWEIGHTS * 512:NWEIGHTS * 512 + D])
            owT2 = consts.tile([CH, D], BF16, tag="owT2")
            nc.sync.dma_start(out=owT2, in_=d_wp.ap()[:, NWEIGHTS * 512 + D:])

            identf = consts.tile([SB, SB], F32, tag="identf")
            make_identity(nc, identf)
            ident = consts.tile([SB, SB], BF16, tag="ident")
            nc.vector.tensor_copy(ident, identf)

            # persistent SBUF tensors
            qk = {w: qk_pool.tile([CH, S], BF16, tag=w, name=w)
                  for w in ("q1", "k1", "q2", "k2")}
            vT = qk_pool.tile([CH, S], BF16, tag="vT")
            va = vaug_pool.tile([SB, 2 * NSB, HD + 1], BF16, tag="va")
            nc.vector.tensor_copy(va[:, :, HD:HD + 1],
                                  ones.to_broadcast((SB, 2 * NSB, 1)))
            oT = o_pool.tile([CH, S], F32, tag="oT")
            xnr = o_pool.tile([CH, S], BF16, tag="xnr")
            U = {(h, a): ubig_pool.tile([HD + 1, S], F32, tag=f"U{h}{a}",
                                         name=f"U{h}{a}")
                 for h in (0, 1) for a in (1, 2)}
            nst = 4
            BST_F = S // nst
            bstats = small.tile([CH, nst, 6], F32, tag="bstats")

            with (
                tc.tile_pool(name="sc", bufs=3, space="PSUM") as sc_pool,
                tc.tile_pool(name="avp", bufs=1, space="PSUM") as av_pool,
            ):
                # ---- projections & v-transpose as small chunks (PE
                # fillers interleaved into the first attention block) ----
                def proj_chunk(w, dst, qb, bias=None):
                    ps = sc_pool.tile([128, QB], F32, tag="sc",
                                      name=f"pj_{w}{qb}")
                    for c in range(4):
                        nc.tensor.matmul(
                            ps, wt[:, WIDX[w], c, :],
                            xt[:, c, qb * QB:(qb + 1) * QB],
                            start=(c == 0), stop=(c == 3))
                    sl = slice(qb * QB, (qb + 1) * QB)
                    if bias is not None:
                        nc.vector.tensor_scalar_add(dst[:, sl], ps, bias)
                    else:
                        nc.vector.tensor_copy(dst[:, sl], ps)

                def vtrans_chunk(grp):
                    pst = sc_pool.tile([128, 4, SB], BF16, tag="sc",
                                       name=f"pv{grp}")
                    for i in range(4):
                        sb = 4 * grp + i
                        nc.tensor.transpose(
                            pst[:, i, :], vT[:, sb * SB:(sb + 1) * SB], ident)
                    nc.vector.tensor_copy(
                        va[:, 8 * grp:8 * grp + 8, 0:HD],
                        pst.rearrange("p i (h m) -> p (i h) m", h=2))

                # ---- attention for one (h, attn): pipelined sc->exp->av,
                # processed in two query-pair chunks so the combine for a
                # chunk overlaps the next chunk's attention ----
                def attention(h, attn, fillers=None, fill_plan=None,
                              chunks=((0, 2 * QB), (2 * QB, 2 * QB))):
                    hs = slice(h * HD, (h + 1) * HD)
                    qT, kT = qk[f"q{attn}"], qk[f"k{attn}"]
                    for pair, (lo, w) in enumerate(chunks):
                        pc = slice(lo, lo + w)
                        av = av_pool.tile([HD + 1, w], F32, tag="av",
                                          name="av")
                        pending = []

                        nsub = w // QB

                        def emit_av(p, av=av, nsub=nsub):
                            ut, kb = p
                            for j in range(nsub):
                                nc.tensor.matmul(
                                    av[:, j * QB:(j + 1) * QB],
                                    va[:, 2 * kb + h, :],
                                    ut[:, j * QB:(j + 1) * QB],
                                    start=(kb == 0), stop=(kb == NKB - 1),
                                )

                        for kb in range(NKB):
                            sct = sc_pool.tile([128, w], F32, tag="sc",
                                               name="sc")
                            for j in range(nsub):
                                q0 = lo + j * QB
                                nc.tensor.matmul(
                                    sct[:, j * QB:(j + 1) * QB],
                                    kT[hs, kb * KB:(kb + 1) * KB],
                                    qT[hs, q0:q0 + QB],
                                    start=True, stop=True,
                                )
                            ut = u_pool.tile([128, w], BF16, tag="u",
                                             name="u")
                            nc.scalar.activation(
                                out=ut, in_=sct,
                                func=mybir.ActivationFunctionType.Exp,
                                scale=1.0 / (HD ** 0.5),
                            )
                            pending.append((ut, kb))
                            if fill_plan is not None:
                                nfill, nflush = fill_plan(pair, kb)
                                for _ in range(nfill):
                                    if fillers:
                                        fillers.popleft()()
                                for _ in range(nflush):
                                    if len(pending) > 1:
                                        emit_av(pending.pop(0))
                            else:
                                if len(pending) > 1:
                                    emit_av(pending.pop(0))
                        for p in pending:
                            emit_av(p)
                        nc.vector.tensor_copy(U[(h, attn)][:, pc], av)
                        if attn == 1:
                            combineA(h, lo, w)
                        else:
                            combineB(h, lo, w)

                t1s = {0: small.tile([HD, S], F32, tag="t1full",
                                     name="t1full")}

                def combineA(h, lo, w):
                    pc = slice(lo, lo + w)
                    rr = small.tile([1, w], F32, tag="rr1", name="rr")
                    nc.vector.reciprocal(out=rr, in_=U[(h, 1)][HD:HD + 1, pc])
                    rb1 = small.tile([HD, w], F32, tag="rb1", name="rb")
                    nc.gpsimd.partition_broadcast(rb1, rr)
                    nc.vector.tensor_mul(t1s[0][:, pc], U[(h, 1)][0:HD, pc],
                                         rb1)

                def combineB(h, lo, w):
                    hs = slice(h * HD, (h + 1) * HD)
                    pc = slice(lo, lo + w)
                    rr = small.tile([1, w], F32, tag="rr2", name="rr")
                    nc.vector.reciprocal(out=rr, in_=U[(h, 2)][HD:HD + 1, pc])
                    rb2 = small.tile([HD, w], F32, tag="rb2", name="rb")
                    nc.gpsimd.partition_broadcast(rb2, rr)
                    t2 = small.tile([HD, w], F32, tag="t2", name="t2")
                    nc.vector.scalar_tensor_tensor(
                        out=t2, in0=U[(h, 2)][0:HD, pc],
                        scalar=neglam[0:HD, h:h + 1], in1=rb2,
                        op0=mybir.AluOpType.mult,
                        op1=mybir.AluOpType.mult,
                    )
                    nc.vector.tensor_add(oT[hs, pc], t1s[0][:, pc], t2)
                    for ii in range(lo // BST_F, (lo + w) // BST_F):
                        nc.vector.bn_stats(
                            out=bstats[hs, ii, :],
                            in_=oT[hs, ii * BST_F:(ii + 1) * BST_F])
                    nc.vector.tensor_copy(xnr[hs, pc], oT[hs, pc])

                # ---- schedule: a 3-chunk prologue starts the exp
                # stream ~3us in; every other projection chunk and the
                # v-transpose ride inside attention(0,1) as PE fillers;
                # attn@V for early key-blocks defers until transposed V
                # chunks exist. ----
                proj_chunk("k1", qk["k1"], 0, k1b)
                proj_chunk("q1", qk["q1"], 0)
                proj_chunk("q1", qk["q1"], 1)
                fillers = deque()
                for qb in (1, 2, 3):
                    fillers.append(lambda qb=qb: proj_chunk("k1", qk["k1"],
                                                            qb, k1b))
                for qb in range(4):
                    fillers.append(lambda qb=qb: proj_chunk("v", vT, qb))
                for g in range(4):
                    fillers.append(lambda g=g: vtrans_chunk(g))
                for qb in (2, 3):
                    fillers.append(lambda qb=qb: proj_chunk("q1", qk["q1"],
                                                            qb))
                for qb in range(4):
                    fillers.append(lambda qb=qb: proj_chunk("q2", qk["q2"],
                                                            qb))
                for qb in range(4):
                    fillers.append(lambda qb=qb: proj_chunk("k2", qk["k2"],
                                                            qb, k2b))

                def fill_plan(pair, kb):
                    it = pair * NKB + kb
                    if it <= 10:
                        return (1, 0)      # 1 filler/iter keeps ACT fed
                    if it <= 16:
                        return (1, 3)      # flush deferred attn@V
                    if it <= 20:
                        return (1, 2)
                    return (0, 2)

                attention(0, 1, fillers, fill_plan)
                attention(0, 2)
                attention(1, 1)
                attention(1, 2, chunks=((0, 2 * QB), (2 * QB, QB),
                                        (3 * QB, QB)))
                # preload the Sqrt activation table while DVE combines
                dummy = small.tile([1, 1], F32, tag="dummy")
                nc.scalar.activation(out=dummy, in_=eps_t,
                                     func=mybir.ActivationFunctionType.Sqrt,
                                     scale=1.0)
                # ---- GroupNorm global stats (inside sc scope: the tiny
                # partition-sum matmuls borrow sc-pool psum, so the final
                # projection pool can open immediately after) ----
                mv = small.tile([CH, 2], F32, tag="mv")
                nc.vector.bn_aggr(out=mv, in_=bstats)
                s12 = small.tile([CH, 2], F32, tag="s12")
                nc.vector.tensor_copy(s12[:, 0:1], mv[:, 0:1])
                nc.vector.scalar_tensor_tensor(
                    out=s12[:, 1:2], in0=mv[:, 0:1], scalar=0.0,
                    in1=mv[:, 0:1], op0=mybir.AluOpType.add,
                    op1=mybir.AluOpType.mult)
                nc.vector.tensor_add(s12[:, 1:2], s12[:, 1:2], mv[:, 1:2])
                st = sc_pool.tile([1, 2], F32, tag="sc", name="st")
                nc.tensor.matmul(st[0:1, 0:1], s12[:, 0:1], ones,
                                 start=True, stop=True)
                nc.tensor.matmul(st[0:1, 1:2], s12[:, 1:2], ones,
                                 start=True, stop=True, skip_group_check=True)
                mu_e2 = small.tile([1, 2], F32, tag="mu_e2")
                nc.vector.tensor_scalar_mul(mu_e2, st[0:1, 0:2], 1.0 / CH)
                sqm = small.tile([1, 1], F32, tag="sqm")
                nc.vector.tensor_mul(sqm, mu_e2[:, 0:1], mu_e2[:, 0:1])
                var = small.tile([1, 1], F32, tag="var")
                nc.vector.tensor_sub(var, mu_e2[:, 1:2], sqm)
                std = small.tile([1, 1], F32, tag="std")
                nc.scalar.activation(out=std, in_=var,
                                     func=mybir.ActivationFunctionType.Sqrt,
                                     bias=eps_t, scale=1.0)
                rstd = small.tile([1, 1], F32, tag="rstd")
                nc.vector.reciprocal(out=rstd, in_=std)
                murstd = small.tile([1, 2], F32, tag="murstd")
                nc.vector.tensor_copy(murstd[:, 0:1], mu_e2[:, 0:1])
                nc.vector.tensor_copy(murstd[:, 1:2], rstd)
                br = small.tile([CH, 2], F32, tag="br")
                nc.gpsimd.partition_broadcast(br, murstd)
                a_t = small.tile([CH, 1], F32, tag="a_t")
                nc.vector.tensor_mul(a_t, br[:, 1:2], gnw)
                amu = small.tile([CH, 1], F32, tag="amu")
                nc.vector.tensor_mul(amu, a_t, br[:, 0:1])
                b_t = small.tile([CH, 1], F32, tag="b_t")
                nc.vector.tensor_sub(b_t, gnb, amu)
                b16 = small.tile([CH, 1], BF16, tag="b16")
                nc.vector.tensor_copy(b16, b_t)
                ybp = sc_pool.tile([1, D], F32, tag="sc", name="ybp")
                nc.tensor.matmul(ybp, b16, owT2, start=True, stop=True,
                                 skip_group_check=True)
                yb = small.tile([1, D], F32, tag="yb")
                nc.vector.tensor_copy(yb, ybp)
                nc.sync.dma_start(out=d_yb.ap(), in_=yb)


            # ---- final projection partial: y = xnr.T @ owT(gnw-folded) ----
            with (
                tc.tile_pool(name="fin", bufs=2, space="PSUM") as fin_pool,
                tc.tile_pool(name="ytp", bufs=2) as yt_pool,
            ):
                half = NSB // 4
                for hf in range(4):
                    ps = fin_pool.tile([SB, half * D], F32, tag="fin",
                                       name="fin")
                    yt = yt_pool.tile([SB, half, D], BF16, tag="yt", name="yt")
                    for i in range(half):
                        sb = hf * half + i
                        nc.tensor.matmul(
                            ps[:, i * D:(i + 1) * D],
                            xnr[:, sb * SB:(sb + 1) * SB],
                            owT,
                            start=True, stop=True,
                        )
                    if hf % 2 == 0:
                        nc.scalar.activation(
                            out=yt,
                            in_=ps.rearrange("p (i d) -> p i d", i=half),
                            func=mybir.ActivationFunctionType.Copy,
                            scale=br[:, 1:2])
                    else:
                        nc.vector.tensor_scalar_mul(
                            yt, ps.rearrange("p (i d) -> p i d", i=half),
                            br[:, 1:2])
                    nc.sync.dma_start(
                        out=d_y.ap().rearrange(
                            "p (hf sb d) -> p hf sb d", hf=4, sb=half)[:, hf],
                        in_=yt)

    nc.compile()
    return nc


def _shard_inputs(inputs):
    import ml_dtypes
    bf = ml_dtypes.bfloat16
    x = np.ascontiguousarray(inputs["x"], np.float32)
    lam = (np.exp(inputs["lambda_q1"] * inputs["lambda_k1"])
           - np.exp(inputs["lambda_q2"] * inputs["lambda_k2"])
           + LAMBDA_INIT).astype(np.float32).reshape(H)
    in_maps = []
    for c in range(N_CORES):
        b, g = divmod(c, 4)
        ch = slice(CH * g, CH * (g + 1))
        # xp[p, c*S+s] = x[b, s, 128c+p]
        xp = np.ascontiguousarray(
            x[b].T.reshape(4, 128, S).transpose(1, 0, 2).reshape(128, 4 * S)
        ).astype(bf)
        wlist = []
        for W in (inputs["Q1_w"], inputs["K1_w"], inputs["Q2_w"],
                  inputs["K2_w"], inputs["V_w"]):
            wT = np.asarray(W)[ch].T  # [512, 128]
            wlist.append(np.ascontiguousarray(
                wT.reshape(4, 128, CH).transpose(1, 0, 2).reshape(128, 512)))
        owT = np.ascontiguousarray(np.asarray(inputs["out_w"])[:, ch].T)
        owTs = owT * np.asarray(inputs["gn_w"])[ch][:, None]
        wp = np.concatenate(wlist + [owTs, owT], axis=1).astype(bf)
        cp = np.stack([
            np.asarray(inputs["K1_b"])[ch],
            np.asarray(inputs["K2_b"])[ch],
            np.asarray(inputs["gn_w"])[ch],
            np.asarray(inputs["gn_b"])[ch],
            np.full(CH, -lam[2 * g], np.float32),
            np.full(CH, -lam[2 * g + 1], np.float32),
        ], axis=1).astype(np.float32)
        in_maps.append({"xp": xp, "wp": wp, "cp": np.ascontiguousarray(cp)})
    return in_maps


def kernel(**inputs):
    inputs = {k: np.asarray(v) for k, v in inputs.items()}
    if "nc" not in _CACHE:
        _CACHE["nc"] = build_program()
    nc = _CACHE["nc"]
    in_maps = _shard_inputs(inputs)
    res = run_bass_kernel_spmd(nc, in_maps, list(range(N_CORES)))
    out_b = np.asarray(inputs["out_b"], np.float32)
    y = np.zeros((B, S, D), np.float32)
    for c in range(N_CORES):
        b = c // 4
        yp = res.results[c]["y_part"].astype(np.float32)
        y[b] += yp.reshape(SB, NSB, D).transpose(1, 0, 2).reshape(S, D)
        y[b] += res.results[c]["yb"].astype(np.float32).reshape(1, D)
    y += out_b[None, None, :]
    return y


# revision 12
# speedup vs baseline: 3.7404x; 1.9107x over previous
"""MultiHeadDiffAttention TRN2 kernel (pipelined, bf16).

Sharding: 8 cores = 2 batches x 4 head-pairs. Core c handles batch c//4 and
heads {2g, 2g+1}, g = c%4; its 128 channels form one GroupNorm group. The
final projection is a partial sum over the core's channels; the host adds
the 4 partials per batch, a per-core bias row (yb), and the output bias.

Design (evolved v1 -> v8 against CoreSim cost-model profiles and HW A/B
probes; HW body time 603us -> ~155us (quiet) depending on terminal load):
  - all matmul operands bf16 (inputs/weights DMA'd bf16; q/k/v/scores path
    bf16). fp32r attention was the original HW bottleneck: fp32r is
    excluded from fast-weight-load, making every 213ns matmul cost ~580ns.
    bf16 keeps rel err ~6.4e-3 (tolerance 2e-2).
  - attention pipelined per (head, attn, 1024-query chunk): per key block,
    2 score MMs (one stationary load) -> one [128,1024] exp on ACT ->
    attn@V MMs deferred one group so PE never waits on ACT. Score psum is
    triple-buffered (2 banks each), attn@V accumulates in a 2-bank tile
    (psum exactly 8 banks). The ones-column appended to V yields the
    softmax denominator for free.
  - every projection chunk and the PE-transpose of V ride inside the
    first attention block as PE fillers (1/iteration; late fillers split
    to 2-matmul half-chunks so each filler burst stays under the exp
    period), so the exp stream starts ~3us in and ACT stays fed.
  - the diff-attn combine (U1/d1 - lam*U2/d2) and GroupNorm bn_stats for a
    chunk run on DVE/Pool under the next chunk's attention; the last
    attention call narrows to 512-col chunks to shrink the exposed tail.
  - GroupNorm's affine is folded away entirely: gn_w is multiplied into
    the output weights HOST-side, the global 1/sigma is applied at the
    psum->bf16 output copies (per-partition broadcast scalar, alternating
    ACT/DVE), and the tiny partition-sum matmuls borrow score-pool psum —
    so the 16 final matmuls unblock right after the last attention chunk
    instead of waiting for the GroupNorm scalar chain. The Sqrt activation
    table is preloaded under the combine.

Timing note: measure with build_program(repeats=N, hw_loop=True) (For_i
device loop) and the slope between two trip counts; host dispatch is
~75-90ms/call and terminal load drifts +/-20%, so only within-process
comparisons are meaningful.
"""

import sys

sys.path.insert(0, "/opt/trn_rl_repo")

import numpy as np
from collections import deque

import concourse.bacc as bacc
import concourse.mybir as mybir
import concourse.tile as tile
from concourse.masks import make_identity
from concourse.bass_utils import run_bass_kernel_spmd

B, S, D = 2, 2048, 512
H = 8
HD = D // H          # 64
CH = 2 * HD          # 128 channels per core (one GroupNorm group)
LAMBDA_INIT = 0.2
EPS = 1e-5
N_CORES = 8

QB = 512             # query chunk (psum bank)
NQB = S // QB        # 4
KB = 128             # key block
NKB = S // KB        # 16
SB = 128             # seq block for final matmul
NSB = S // SB        # 16

F32 = mybir.dt.float32
F32R = mybir.dt.float32r
BF16 = mybir.dt.bfloat16
NWEIGHTS = 5
WIDX = {"q1": 0, "k1": 1, "q2": 2, "k2": 3, "v": 4}

_CACHE = {}

from contextlib import nullcontext


def build_program(repeats=1, hw_loop=False):
    nc = bacc.Bacc("TRN2", target_bir_lowering=False, debug=False)

    # ---- external I/O (packed per-partition-contiguous host layouts) ----
    # xp[p, c*S + s] = x[b, s, 128c+p]              (bf16)
    d_xp = nc.declare_dram_parameter("xp", [128, 4 * S], BF16, isOutput=False)
    # wp[p, w*512 + c*128 + m] = W_w[ch0+m, 128c+p]; then owT[p, d] tail
    d_wp = nc.declare_dram_parameter("wp", [128, NWEIGHTS * 512 + 2 * D],
                                     BF16, isOutput=False)
    # cp[p, :] = [k1b, k2b, gnw, gnb, neglam0, neglam1]
    d_cp = nc.declare_dram_parameter("cp", [CH, 6], F32, isOutput=False)
    # yp[p, sb*D + d] = y_part[128*sb+p, d]         (bf16)
    d_y = nc.declare_dram_parameter("y_part", [SB, NSB * D], BF16,
                                    isOutput=True)
    # yb[0, d] = (gn_b_eff . owT)[d] — constant row added host-side
    d_yb = nc.declare_dram_parameter("yb", [1, D], F32, isOutput=True)

    with tile.TileContext(nc) as tc:
     with (tc.For_i(0, repeats) if hw_loop else nullcontext()):
      for _rep in range(1 if hw_loop else repeats):
        with (
            tc.tile_pool(name="consts", bufs=1) as consts,
            tc.tile_pool(name="qk", bufs=1) as qk_pool,
            tc.tile_pool(name="vaug", bufs=1) as vaug_pool,
            tc.tile_pool(name="xtp", bufs=1) as xt_pool,
            tc.tile_pool(name="upool", bufs=13) as u_pool,
            tc.tile_pool(name="ubig", bufs=1) as ubig_pool,
            tc.tile_pool(name="opool", bufs=1) as o_pool,
            tc.tile_pool(name="small", bufs=1) as small,
        ):
            # ---- constants / packed inputs ----
            ones = consts.tile([128, 1], F32, tag="ones")
            nc.vector.memset(ones, 1.0)
            eps_t = consts.tile([1, 1], F32, tag="eps")
            nc.vector.memset(eps_t, EPS)
            cp = consts.tile([CH, 6], F32, tag="cp")
            nc.sync.dma_start(out=cp, in_=d_cp.ap())
            k1b, k2b = cp[:, 0:1], cp[:, 1:2]
            gnw, gnb = cp[:, 2:3], cp[:, 3:4]
            neglam = cp[:, 4:6]

            wt = consts.tile([128, NWEIGHTS, 4, CH], BF16, tag="wt")
            # q1+k1 weight slices first: they gate the prologue projections
            nc.sync.dma_start(
                out=wt[:, 0:2],
                in_=d_wp.ap()[:, 0:1024].rearrange(
                    "p (w c m) -> p w c m", w=2, c=4))

            xt = xt_pool.tile([128, 4, S], BF16, tag="xt")
            for half in range(2):
                hsl = slice(half * (S // 2), (half + 1) * (S // 2))
                for c in range(4):
                    nc.sync.dma_start(
                        out=xt[:, c, hsl],
                        in_=d_xp.ap()[:, c * S + half * (S // 2):
                                      c * S + (half + 1) * (S // 2)])
                if half == 0:
                    nc.sync.dma_start(
                        out=wt[:, 2:NWEIGHTS],
                        in_=d_wp.ap()[:, 1024:NWEIGHTS * 512].rearrange(
                            "p (w c m) -> p w c m", w=3, c=4))

            # owT carries gn_w pre-folded (host); owT2 is unscaled (yb row)
            owT = consts.tile([CH, D], BF16, tag="owT")
            nc.sync.dma_start(
                out=owT, in_=d_wp.ap()[:, NWEIGHTS * 512:NWEIGHTS * 512 + D])
            owT2 = consts.tile([CH, D], BF16, tag="owT2")
            nc.sync.dma_start(out=owT2, in_=d_wp.ap()[:, NWEIGHTS * 512 + D:])

            identf = consts.tile([SB, SB], F32, tag="identf")
            make_identity(nc, identf)
            ident = consts.tile([SB, SB], BF16, tag="ident")
            nc.vector.tensor_copy(ident, identf)

            # persistent SBUF tensors
            qk = {w: qk_pool.tile([CH, S], BF16, tag=w, name=w)
                  for w in ("q1", "k1", "q2", "k2")}
            vT = qk_pool.tile([CH, S], BF16, tag="vT")
            va = vaug_pool.tile([SB, 2 * NSB, HD + 1], BF16, tag="va")
            nc.vector.tensor_copy(va[:, :, HD:HD + 1],
                                  ones.to_broadcast((SB, 2 * NSB, 1)))
            oT = o_pool.tile([CH, S], F32, tag="oT")
            xnr = o_pool.tile([CH, S], BF16, tag="xnr")
            U = {(h, a): ubig_pool.tile([HD + 1, S], F32, tag=f"U{h}{a}",
                                         name=f"U{h}{a}")
                 for h in (0, 1) for a in (1, 2)}
            nst = 4
            BST_F = S // nst
            bstats = small.tile([CH, nst, 6], F32, tag="bstats")

            with (
                tc.tile_pool(name="sc", bufs=3, space="PSUM") as sc_pool,
                tc.tile_pool(name="avp", bufs=1, space="PSUM") as av_pool,
            ):
                # ---- projections & v-transpose as small chunks (PE
                # fillers interleaved into the first attention block) ----
                pj_live = {}

                def proj_chunk(w, dst, qb, bias=None, crange=(0, 4)):
                    key = (w, qb)
                    if key not in pj_live:
                        pj_live[key] = sc_pool.tile([128, QB], F32, tag="sc",
                                                    name=f"pj_{w}{qb}")
                    ps = pj_live[key]
                    for c in range(*crange):
                        nc.tensor.matmul(
                            ps, wt[:, WIDX[w], c, :],
                            xt[:, c, qb * QB:(qb + 1) * QB],
                            start=(c == 0), stop=(c == 3))
                    if crange[1] == 4:
                        del pj_live[key]
                        sl = slice(qb * QB, (qb + 1) * QB)
                        if bias is not None:
                            nc.vector.tensor_scalar_add(dst[:, sl], ps, bias)
                        else:
                            nc.vector.tensor_copy(dst[:, sl], ps)

                def vtrans_chunk(grp):
                    pst = sc_pool.tile([128, 4, SB], BF16, tag="sc",
                                       name=f"pv{grp}")
                    for i in range(4):
                        sb = 4 * grp + i
                        nc.tensor.transpose(
                            pst[:, i, :], vT[:, sb * SB:(sb + 1) * SB], ident)
                    nc.vector.tensor_copy(
                        va[:, 8 * grp:8 * grp + 8, 0:HD],
                        pst.rearrange("p i (h m) -> p (i h) m", h=2))

                # ---- attention for one (h, attn): pipelined sc->exp->av,
                # processed in two query-pair chunks so the combine for a
                # chunk overlaps the next chunk's attention ----
                def attention(h, attn, fillers=None, fill_plan=None,
                              chunks=((0, 2 * QB), (2 * QB, 2 * QB))):
                    hs = slice(h * HD, (h + 1) * HD)
                    qT, kT = qk[f"q{attn}"], qk[f"k{attn}"]
                    for pair, (lo, w) in enumerate(chunks):
                        pc = slice(lo, lo + w)
                        av = av_pool.tile([HD + 1, w], F32, tag="av",
                                          name="av")
                        pending = []

                        nsub = w // QB

                        def emit_av(p, av=av, nsub=nsub):
                            ut, kb = p
                            for j in range(nsub):
                                nc.tensor.matmul(
                                    av[:, j * QB:(j + 1) * QB],
                                    va[:, 2 * kb + h, :],
                                    ut[:, j * QB:(j + 1) * QB],
                                    start=(kb == 0), stop=(kb == NKB - 1),
                                )

                        for kb in range(NKB):
                            sct = sc_pool.tile([128, w], F32, tag="sc",
                                               name="sc")
                            for j in range(nsub):
                                q0 = lo + j * QB
                                nc.tensor.matmul(
                                    sct[:, j * QB:(j + 1) * QB],
                                    kT[hs, kb * KB:(kb + 1) * KB],
                                    qT[hs, q0:q0 + QB],
                                    start=True, stop=True,
                                )
                            ut = u_pool.tile([128, w], BF16, tag="u",
                                             name="u")
                            nc.scalar.activation(
                                out=ut, in_=sct,
                                func=mybir.ActivationFunctionType.Exp,
                                scale=1.0 / (HD ** 0.5),
                            )
                            pending.append((ut, kb))
                            if fill_plan is not None:
                                nfill, nflush = fill_plan(pair, kb)
                                for _ in range(nfill):
                                    if fillers:
                                        fillers.popleft()()
                                for _ in range(nflush):
                                    if len(pending) > 1:
                                        emit_av(pending.pop(0))
                            else:
                                if len(pending) > 1:
                                    emit_av(pending.pop(0))
                        for p in pending:
                            emit_av(p)
                        nc.vector.tensor_copy(U[(h, attn)][:, pc], av)
                        if attn == 1:
                            combineA(h, lo, w)
                        else:
                            combineB(h, lo, w)

                t1s = {0: small.tile([HD, S], F32, tag="t1full",
                                     name="t1full")}

                def combineA(h, lo, w):
                    pc = slice(lo, lo + w)
                    rr = small.tile([1, w], F32, tag="rr1", name="rr")
                    nc.vector.reciprocal(out=rr, in_=U[(h, 1)][HD:HD + 1, pc])
                    rb1 = small.tile([HD, w], F32, tag="rb1", name="rb")
                    nc.gpsimd.partition_broadcast(rb1, rr)
                    nc.vector.tensor_mul(t1s[0][:, pc], U[(h, 1)][0:HD, pc],
                                         rb1)

                def combineB(h, lo, w):
                    hs = slice(h * HD, (h + 1) * HD)
                    pc = slice(lo, lo + w)
                    rr = small.tile([1, w], F32, tag="rr2", name="rr")
                    nc.vector.reciprocal(out=rr, in_=U[(h, 2)][HD:HD + 1, pc])
                    rb2 = small.tile([HD, w], F32, tag="rb2", name="rb")
                    nc.gpsimd.partition_broadcast(rb2, rr)
                    t2 = small.tile([HD, w], F32, tag="t2", name="t2")
                    nc.vector.scalar_tensor_tensor(
                        out=t2, in0=U[(h, 2)][0:HD, pc],
                        scalar=neglam[0:HD, h:h + 1], in1=rb2,
                        op0=mybir.AluOpType.mult,
                        op1=mybir.AluOpType.mult,
                    )
                    nc.vector.tensor_add(oT[hs, pc], t1s[0][:, pc], t2)
                    for ii in range(lo // BST_F, (lo + w) // BST_F):
                        nc.vector.bn_stats(
                            out=bstats[hs, ii, :],
                            in_=oT[hs, ii * BST_F:(ii + 1) * BST_F])
                    nc.vector.tensor_copy(xnr[hs, pc], oT[hs, pc])

                # ---- schedule: a 3-chunk prologue starts the exp
                # stream ~3us in; every other projection chunk and the
                # v-transpose ride inside attention(0,1) as PE fillers;
                # attn@V for early key-blocks defers until transposed V
                # chunks exist. ----
                proj_chunk("k1", qk["k1"], 0, k1b)
                proj_chunk("q1", qk["q1"], 0)
                proj_chunk("q1", qk["q1"], 1)
                fillers = deque()
                for qb in (1, 2, 3):
                    fillers.append(lambda qb=qb: proj_chunk("k1", qk["k1"],
                                                            qb, k1b))
                for qb in range(4):
                    fillers.append(lambda qb=qb: proj_chunk("v", vT, qb))
                for g in range(4):
                    fillers.append(lambda g=g: vtrans_chunk(g))
                def half_chunks(w, dst, qbs, bias=None):
                    for qb in qbs:
                        fillers.append(
                            lambda qb=qb: proj_chunk(w, dst, qb, bias,
                                                     crange=(0, 2)))
                        fillers.append(
                            lambda qb=qb: proj_chunk(w, dst, qb, bias,
                                                     crange=(2, 4)))
                half_chunks("q1", qk["q1"], (2, 3))
                half_chunks("q2", qk["q2"], range(4))
                half_chunks("k2", qk["k2"], range(4), k2b)

                def fill_plan(pair, kb):
                    it = pair * NKB + kb
                    if it <= 10:
                        return (1, 0)      # 1 filler/iter keeps ACT fed
                    if it <= 16:
                        return (1, 3)      # flush deferred attn@V
                    if it <= 30:
                        return (1, 2)      # late half-fillers, 1/iter
                    return (0, 2)

                attention(0, 1, fillers, fill_plan)
                attention(0, 2)
                attention(1, 1)
                attention(1, 2, chunks=((0, 2 * QB), (2 * QB, QB),
                                        (3 * QB, QB)))
                # preload the Sqrt activation table while DVE combines
                dummy = small.tile([1, 1], F32, tag="dummy")
                nc.scalar.activation(out=dummy, in_=eps_t,
                                     func=mybir.ActivationFunctionType.Sqrt,
                                     scale=1.0)
                # ---- GroupNorm global stats (inside sc scope: the tiny
                # partition-sum matmuls borrow sc-pool psum, so the final
                # projection pool can open immediately after) ----
                mv = small.tile([CH, 2], F32, tag="mv")
                nc.vector.bn_aggr(out=mv, in_=bstats)
                s12 = small.tile([CH, 2], F32, tag="s12")
                nc.vector.tensor_copy(s12[:, 0:1], mv[:, 0:1])
                nc.vector.scalar_tensor_tensor(
                    out=s12[:, 1:2], in0=mv[:, 0:1], scalar=0.0,
                    in1=mv[:, 0:1], op0=mybir.AluOpType.add,
                    op1=mybir.AluOpType.mult)
                nc.vector.tensor_add(s12[:, 1:2], s12[:, 1:2], mv[:, 1:2])
                st = sc_pool.tile([1, 2], F32, tag="sc", name="st")
                nc.tensor.matmul(st[0:1, 0:1], s12[:, 0:1], ones,
                                 start=True, stop=True)
                nc.tensor.matmul(st[0:1, 1:2], s12[:, 1:2], ones,
                                 start=True, stop=True, skip_group_check=True)
                mu_e2 = small.tile([1, 2], F32, tag="mu_e2")
                nc.vector.tensor_scalar_mul(mu_e2, st[0:1, 0:2], 1.0 / CH)
                sqm = small.tile([1, 1], F32, tag="sqm")
                nc.vector.tensor_mul(sqm, mu_e2[:, 0:1], mu_e2[:, 0:1])
                var = small.tile([1, 1], F32, tag="var")
                nc.vector.tensor_sub(var, mu_e2[:, 1:2], sqm)
                std = small.tile([1, 1], F32, tag="std")
                nc.scalar.activation(out=std, in_=var,
                                     func=mybir.ActivationFunctionType.Sqrt,
                                     bias=eps_t, scale=1.0)
                rstd = small.tile([1, 1], F32, tag="rstd")
                nc.vector.reciprocal(out=rstd, in_=std)
                murstd = small.tile([1, 2], F32, tag="murstd")
                nc.vector.tensor_copy(murstd[:, 0:1], mu_e2[:, 0:1])
                nc.vector.tensor_copy(murstd[:, 1:2], rstd)
                br = small.tile([CH, 2], F32, tag="br")
                nc.gpsimd.partition_broadcast(br, murstd)
                a_t = small.tile([CH, 1], F32, tag="a_t")
                nc.vector.tensor_mul(a_t, br[:, 1:2], gnw)
                amu = small.tile([CH, 1], F32, tag="amu")
                nc.vector.tensor_mul(amu, a_t, br[:, 0:1])
                b_t = small.tile([CH, 1], F32, tag="b_t")
                nc.vector.tensor_sub(b_t, gnb, amu)
                b16 = small.tile([CH, 1], BF16, tag="b16")
                nc.vector.tensor_copy(b16, b_t)
                ybp = sc_pool.tile([1, D], F32, tag="sc", name="ybp")
                nc.tensor.matmul(ybp, b16, owT2, start=True, stop=True,
                                 skip_group_check=True)
                yb = small.tile([1, D], F32, tag="yb")
                nc.vector.tensor_copy(yb, ybp)
                nc.sync.dma_start(out=d_yb.ap(), in_=yb)


            # ---- final projection partial: y = xnr.T @ owT(gnw-folded) ----
            with (
                tc.tile_pool(name="fin", bufs=2, space="PSUM") as fin_pool,
                tc.tile_pool(name="ytp", bufs=2) as yt_pool,
            ):
                half = NSB // 4
                for hf in range(4):
                    ps = fin_pool.tile([SB, half * D], F32, tag="fin",
                                       name="fin")
                    yt = yt_pool.tile([SB, half, D], BF16, tag="yt", name="yt")
                    for i in range(half):
                        sb = hf * half + i
                        nc.tensor.matmul(
                            ps[:, i * D:(i + 1) * D],
                            xnr[:, sb * SB:(sb + 1) * SB],
                            owT,
                            start=True, stop=True,
                        )
                    if hf % 2 == 0:
                        nc.scalar.activation(
                            out=yt,
                            in_=ps.rearrange("p (i d) -> p i d", i=half),
                            func=mybir.ActivationFunctionType.Copy,
                            scale=br[:, 1:2])
                    else:
                        nc.vector.tensor_scalar_mul(
                            yt, ps.rearrange("p (i d) -> p i d", i=half),
                            br[:, 1:2])
                    nc.sync.dma_start(
                        out=d_y.ap().rearrange(
                            "p (hf sb d) -> p hf sb d", hf=4, sb=half)[:, hf],
                        in_=yt)

    nc.compile()
    return nc


def _shard_inputs(inputs):
    import ml_dtypes
    bf = ml_dtypes.bfloat16
    x = np.ascontiguousarray(inputs["x"], np.float32)
    lam = (np.exp(inputs["lambda_q1"] * inputs["lambda_k1"])
           - np.exp(inputs["lambda_q2"] * inputs["lambda_k2"])
           + LAMBDA_INIT).astype(np.float32).reshape(H)
    in_maps = []
    for c in range(N_CORES):
        b, g = divmod(c, 4)
        ch = slice(CH * g, CH * (g + 1))
        # xp[p, c*S+s] = x[b, s, 128c+p]
        xp = np.ascontiguousarray(
            x[b].T.reshape(4, 128, S).transpose(1, 0, 2).reshape(128, 4 * S)
        ).astype(bf)
        wlist = []
        for W in (inputs["Q1_w"], inputs["K1_w"], inputs["Q2_w"],
                  inputs["K2_w"], inputs["V_w"]):
            wT = np.asarray(W)[ch].T  # [512, 128]
            wlist.append(np.ascontiguousarray(
                wT.reshape(4, 128, CH).transpose(1, 0, 2).reshape(128, 512)))
        owT = np.ascontiguousarray(np.asarray(inputs["out_w"])[:, ch].T)
        owTs = owT * np.asarray(inputs["gn_w"])[ch][:, None]
        wp = np.concatenate(wlist + [owTs, owT], axis=1).astype(bf)
        cp = np.stack([
            np.asarray(inputs["K1_b"])[ch],
            np.asarray(inputs["K2_b"])[ch],
            np.asarray(inputs["gn_w"])[ch],
            np.asarray(inputs["gn_b"])[ch],
            np.full(CH, -lam[2 * g], np.float32),
            np.full(CH, -lam[2 * g + 1], np.float32),
        ], axis=1).astype(np.float32)
        in_maps.append({"xp": xp, "wp": wp, "cp": np.ascontiguousarray(cp)})
    return in_maps


def kernel(**inputs):
    inputs = {k: np.asarray(v) for k, v in inputs.items()}
    if "nc" not in _CACHE:
        _CACHE["nc"] = build_program()
    nc = _CACHE["nc"]
    in_maps = _shard_inputs(inputs)
    res = run_bass_kernel_spmd(nc, in_maps, list(range(N_CORES)))
    out_b = np.asarray(inputs["out_b"], np.float32)
    y = np.zeros((B, S, D), np.float32)
    for c in range(N_CORES):
        b = c // 4
        yp = res.results[c]["y_part"].astype(np.float32)
        y[b] += yp.reshape(SB, NSB, D).transpose(1, 0, 2).reshape(S, D)
        y[b] += res.results[c]["yb"].astype(np.float32).reshape(1, D)
    y += out_b[None, None, :]
    return y
